# revision 64
# baseline (speedup 1.0000x reference)
"""MLA q/k/v projection kernel for Trainium2 (8 NeuronCores, token-data-parallel).

Self-contained: hardcodes the problem shapes from nn_MLA_81106162418389.
  hidden_state [2, 4096, 2048] f32 -> out [2, 16, 4096, 512] f32
Strategy: shard the 8192 tokens over 8 cores (1024 each); replicate weights.
All matmul operands in bf16; single persistent pool structure (no phase
barriers); PSUM = 3x[P,2,512] + 2x[1,512] ring (8 banks exactly).
"""
import sys
sys.path.insert(0, "/opt/trn_rl_repo")

import numpy as np
import ml_dtypes

import concourse.bass as bass
import concourse.tile as tile
from concourse import bacc, mybir
from concourse import bass2jax
from concourse.masks import make_identity


# ---- problem constants ----
HID, QK_NOPE, QK_ROPE, Q_LR, KV_LR, H, V_DIM = 2048, 128, 64, 768, 512, 16, 128
QK_HEAD = QK_NOPE + QK_ROPE           # 192
OUT_C = 2 * QK_HEAD + V_DIM           # 512
B, S = 2, 4096
THETA = 10000.0
EPS = 1e-5

N_CORES = 8
T = (B * S) // N_CORES                # 1024 tokens per core
P = 128
TCN = T // P                          # 8 token chunks
NT = 2                                # 512-wide token tiles for phase 1
KO = HID // P                         # 16 k-chunks for a-proj
ROQ = Q_LR // P                       # 6 r-chunks for q up-proj
ROKV = KV_LR // P                     # 4 r-chunks for kv up-proj
HH = H // 2                           # 8 heads per half-pass
QW = HH * QK_HEAD                     # 1536 q cols per half
KW = HH * (QK_NOPE + V_DIM)           # 2048 kv cols per half
# combined a-proj output columns: q 0:768 | kv 768:1280 | rope 1280:1344 |
# mu_q 1344 | mu_kv 1345 | pad to 11 chunks of 128.  The mu columns hold
# rowsum(W)/dim so the LN means come out of the same matmuls for free.
NFC = 11
CW = NFC * P                          # 1408

F32 = mybir.dt.float32
BF16 = mybir.dt.bfloat16
FP8 = mybir.dt.float8e4
DR = mybir.MatmulPerfMode.DoubleRow
AF = mybir.ActivationFunctionType
OP = mybir.AluOpType
# first 256 contraction dims of each up-proj run as fp8 DoubleRow (2x PE
# rate); weights are pre-scaled x8 on the host so w*8 ~ N(0,0.16) clears
# the e4m3 subnormal band, and 1/8 is folded into istd via the Sqrt scale.
WSC = 8.0


def _build(n_repeats=1, has_qb_bias=False, has_kvb_bias=False):
    nc = bacc.Bacc("TRN2", target_bir_lowering=False, debug=False,
                   num_devices=N_CORES)

    xT_d = nc.dram_tensor("xT", [HID, T], BF16, kind="ExternalInput").ap()
    waT_d = nc.dram_tensor("waT", [HID, CW], BF16, kind="ExternalInput").ap()
    wqb8_d = nc.dram_tensor("wqb8", [P, 2, H * QK_HEAD], FP8,
                            kind="ExternalInput").ap()
    wqb16_d = nc.dram_tensor("wqb16", [P, ROQ - 2, H * QK_HEAD], BF16,
                             kind="ExternalInput").ap()
    wkvb8_d = nc.dram_tensor("wkvb8", [P, 2, H * (QK_NOPE + V_DIM)], FP8,
                             kind="ExternalInput").ap()
    wkvb16_d = nc.dram_tensor("wkvb16", [P, ROKV - 2, H * (QK_NOPE + V_DIM)],
                              BF16, kind="ExternalInput").ap()
    biasA_d = nc.dram_tensor("biasA", [P, NFC], F32, kind="ExternalInput").ap()
    cos_d = nc.dram_tensor("cosb", [P, TCN, QK_ROPE], F32,
                           kind="ExternalInput").ap()
    sin_d = nc.dram_tensor("sinb", [P, TCN, QK_ROPE], F32,
                           kind="ExternalInput").ap()
    bqb_d = bkvb_d = None
    if has_qb_bias:
        bqb_d = nc.dram_tensor("bqb", [H * QK_HEAD], F32, kind="ExternalInput").ap()
    if has_kvb_bias:
        bkvb_d = nc.dram_tensor("bkvb", [H * (QK_NOPE + V_DIM)], F32,
                                kind="ExternalInput").ap()
    out_d = nc.dram_tensor("out", [H, T, OUT_C], F32, kind="ExternalOutput").ap()

    for _ in range(n_repeats):
        _emit_once(nc, xT_d, waT_d, wqb8_d, wqb16_d, wkvb8_d, wkvb16_d,
                   biasA_d, cos_d, sin_d, bqb_d, bkvb_d, out_d)
    nc.compile()
    return nc


def _emit_once(nc, xT_d, waT_d, wqb8_d, wqb16_d, wkvb8_d, wkvb16_d,
               biasA_d, cos_d, sin_d, bqb_d, bkvb_d, out_d):
    with tile.TileContext(nc) as tc:
        with tc.tile_pool(name="pp", bufs=1) as pp, \
             tc.tile_pool(name="ws", bufs=1) as ws, \
             tc.tile_pool(name="psp", bufs=1, space="PSUM") as psp:

            # ---- persistent smalls (x0/wa0 jump the DMA queue below) ----
            biasA_sb = pp.tile([P, NFC], F32)
            cos_sb = pp.tile([P, TCN, QK_ROPE], F32)
            sin_sb = pp.tile([P, TCN, QK_ROPE], F32)
            bqb_bc = bkvb_bc = None
            if bqb_d is not None:
                b1 = pp.tile([1, H * QK_HEAD], F32)
                nc.sync.dma_start(b1[:], bqb_d[None, :])
                bqb_bc = pp.tile([P, H * QK_HEAD], F32)
                nc.gpsimd.partition_broadcast(bqb_bc[:], b1[:])
            if bkvb_d is not None:
                b2 = pp.tile([1, H * (QK_NOPE + V_DIM)], F32)
                nc.sync.dma_start(b2[:], bkvb_d[None, :])
                bkvb_bc = pp.tile([P, H * (QK_NOPE + V_DIM)], F32)
                nc.gpsimd.partition_broadcast(bkvb_bc[:], b2[:])

            ones_b = pp.tile([P, 1], BF16)
            nc.gpsimd.memset(ones_b[:], 1.0)
            warm = pp.tile([P, P], BF16)
            nc.gpsimd.memset(warm[:], 0.0)
            # Sqrt runs with scale=WSC^2 so istd comes out as 1/(WSC*std),
            # compensating the x WSC pre-scale baked into the up-proj weights
            eps_t = pp.tile([1, 1], F32)
            nc.gpsimd.memset(eps_t[:], WSC * WSC * EPS)
            ident = pp.tile([P, P], F32)
            make_identity(nc, ident[:])

            # ---- persistent activations ----
            q_cT = pp.tile([P, ROQ, T], BF16)
            kv_cT = pp.tile([P, ROKV, T], BF16)
            q_cT8 = pp.tile([P, 2, T], FP8)
            kv_cT8 = pp.tile([P, 2, T], FP8)
            # rows 0:64 = k_rope; row 64 = mu_q; row 96 = mu_kv (single-
            # partition accesses must start at a 32-partition boundary)
            rope_mu = pp.tile([P, T], F32)
            krope_t = pp.tile([P, TCN, QK_ROPE], F32)
            krot = pp.tile([P, TCN, QK_ROPE], F32)
            nbc_q = pp.tile([P, T], BF16)
            nbc_kv = pp.tile([P, T], BF16)
            istq_t = pp.tile([P, TCN], F32)
            istkv_t = pp.tile([P, TCN], F32)
            istq_row = pp.tile([1, T], F32)
            istkv_row = pp.tile([1, T], F32)

            # ---- bulk loads, in consumption order on one queue ----
            # The x/wa stream needs ~320 GB/s for the first ~20us to keep
            # mm1 fed: nothing else may share HBM until it is done.
            # q-columns ride the critical front stream with x; the kv/rope/mu
            # columns (needed ~20us later) load in a second stream so the
            # front stays under the ~400 GB/s HBM ceiling.
            # k=0 loads split in halves so the very first matmul can start
            # after ~320KB instead of ~450KB
            x0h = [ws.tile([P, 512], BF16, tag="x0h", bufs=2, name=f"x0_{h}")
                   for h in range(2)]
            x_t, waq_t, wakv_t = [], [], []
            for k in range(KO):
                xt = ws.tile([P, T], BF16, tag="x", bufs=KO, name=f"x_{k}")
                wt = ws.tile([P, Q_LR], BF16, tag="waq", bufs=KO,
                             name=f"waq_{k}")
                if k == 0:
                    nc.sync.dma_start(wt[:], waT_d[0:P, 0:Q_LR])
                    for h in range(2):
                        nc.sync.dma_start(x0h[h][:],
                                          xT_d[0:P, h * 512:(h + 1) * 512])
                    nc.sync.dma_start(biasA_sb[:], biasA_d[:])
                else:
                    nc.sync.dma_start(xt[:], xT_d[k * P:(k + 1) * P, :])
                    nc.sync.dma_start(wt[:], waT_d[k * P:(k + 1) * P, 0:Q_LR])
                x_t.append(xt)
                waq_t.append(wt)
            for k in range(KO):
                wt = ws.tile([P, CW - Q_LR], BF16, tag="wakv", bufs=KO,
                             name=f"wakv_{k}")
                nc.sync.dma_start(wt[:], waT_d[k * P:(k + 1) * P, Q_LR:CW])
                wakv_t.append(wt)
            nc.sync.dma_start(cos_sb[:], cos_d[:])
            nc.sync.dma_start(sin_sb[:], sin_d[:])
            wq8 = ws.tile([P, 2, H * QK_HEAD], FP8, tag="wq8", bufs=1,
                          name="wq8")
            nc.sync.dma_start(wq8[:], wqb8_d)
            wq16 = ws.tile([P, ROQ - 2, H * QK_HEAD], BF16, tag="wq", bufs=1,
                           name="wq16")
            nc.sync.dma_start(wq16[:], wqb16_d)
            wkv8 = ws.tile([P, 2, H * (QK_NOPE + V_DIM)], FP8, tag="wkv8",
                           bufs=1, name="wkv8")
            nc.sync.dma_start(wkv8[:], wkvb8_d)
            # bf16 kv up-proj rows (ro 2,3) ride the freed x slots; their
            # slot-blocked waits live on the idle gpsimd queue so the sync
            # queue is clean for phase-2 stores.  The dma_starts are EMITTED
            # later (after the q-LN broadcasts) so the gpsimd engine stream
            # does not stall the LN mean-broadcasts behind the slot waits.
            wkv_t = {}
            for half in range(2):
                for ro2 in range(ROKV - 2):
                    for piece in range(2):
                        t = ws.tile([P, T], BF16, tag="x", bufs=KO,
                                    name=f"wkv_{half}_{ro2}_{piece}")
                        wkv_t[(half, ro2, piece)] = t

            def _wkv_loads():
                for half in range(2):
                    c0kv = half * KW
                    for ro2 in range(ROKV - 2):
                        for piece in range(2):
                            nc.gpsimd.dma_start(
                                wkv_t[(half, ro2, piece)][:],
                                wkvb16_d[:, ro2,
                                         c0kv + piece * 1024:
                                         c0kv + (piece + 1) * 1024])

            # ---- PE p-state warmup during the DMA lead-in ----
            warm_ps = psp.tile([1, 512], F32, tag="st", bufs=2, name="warm")
            for _ in range(24):
                nc.tensor.matmul(warm_ps[:, 0:P], ones_b[:], warm[:, :],
                                 start=True, stop=True)

            # ================= phase 1: a-projections + LN =================
            def _ln_sq(src, nfc, which):
                # squares summed across r-chunks on DVE, so each LN stat is
                # a single PE matmul instead of nfc accumulating ones
                sqs = {}
                for nt in range(NT):
                    nts = slice(nt * 512, (nt + 1) * 512)
                    acc = ws.tile([P, 512], BF16, tag="ssum", bufs=2,
                                  name=f"ss_{which}_{nt}")
                    nc.vector.tensor_tensor(acc[:], src[:, 0, nts],
                                            src[:, 0, nts], OP.mult)
                    for fc in range(1, nfc):
                        sq = ws.tile([P, 512], BF16, tag="sq", bufs=2,
                                     name=f"sq_{which}_{nt}_{fc}")
                        nc.vector.tensor_tensor(sq[:], src[:, fc, nts],
                                                src[:, fc, nts], OP.mult)
                        nc.vector.tensor_tensor(acc[:], acc[:], sq[:],
                                                OP.add)
                    sqs[nt] = acc
                return sqs

            def _ln(src, src8, nfc, dim, nbc, istd_row, sqs, mu_part):
                # mean-subtract src in place (mu came out of the a-proj's mu
                # column); 1/(WSC*std) goes to istd_row, applied later as a
                # per-partition scale on the phase-2 copies.  The first two
                # r-chunks are written as fp8 for the DoubleRow matmuls.
                # Subtracts run FIRST in the DVE stream: the fp8 copies gate
                # the phase-2 DoubleRow matmuls, the istd chain does not.
                for nt in range(NT):
                    nts = slice(nt * 512, (nt + 1) * 512)
                    mu = rope_mu[mu_part:mu_part + 1, nts]
                    nh = ws.tile([1, 512], BF16, tag="nh", bufs=2,
                                 name=f"nh_{nt}")
                    nc.vector.tensor_scalar_mul(nh[:], mu, -1.0)
                    nc.gpsimd.partition_broadcast(nbc[:, nts], nh[:])
                for fc in range(nfc):
                    dst = src8[:, fc, :] if fc < 2 else src[:, fc, :]
                    nc.vector.tensor_tensor(dst, src[:, fc, :],
                                            nbc[:], OP.add)
                for nt in range(NT):
                    nts = slice(nt * 512, (nt + 1) * 512)
                    ps_q = psp.tile([1, 512], F32, tag="st", bufs=2,
                                    name=f"psq_{nt}")
                    nc.tensor.matmul(ps_q[:], ones_b[:], sqs[nt][:],
                                     start=True, stop=True)
                    mu = rope_mu[mu_part:mu_part + 1, nts]
                    istd = istd_row[:, nts]
                    nc.vector.tensor_tensor(istd, mu, mu, OP.mult)
                    nc.vector.scalar_tensor_tensor(
                        istd, ps_q[:], 1.0 / dim, istd,
                        OP.mult, OP.subtract)
                    nc.scalar.activation(istd, istd, AF.Sqrt,
                                         bias=eps_t[:, 0:1], scale=WSC * WSC)
                    nc.vector.reciprocal_approx_fast(istd, istd)

            def _ist_tpose(ist_t, istd_row, name):
                # istd [1, T] -> token-major [P, TCN] via PE transposes of
                # [1,128] segments (no DRAM round-trip, no queue blocking).
                # Emitted at a PE point where istd_row is long since ready.
                ptile = psp.tile([P, NT, 512], F32, tag="big", bufs=3,
                                 name=f"istp_{name}")
                for tc in range(TCN):
                    nc.tensor.transpose(ptile[:, 0, tc:tc + 1],
                                        istd_row[:, tc * P:(tc + 1) * P],
                                        ident[:1, :1])
                nc.scalar.copy(ist_t[:], ptile[:, 0, 0:TCN])

            # combined a-proj over 11 fc chunks; mu columns ride chunk 10
            kv_sqs = None
            for fcs in ((0, 1, 2), (3, 4, 5), (10,), (6, 7, 8), (9,)):
                tiles = {}
                for fc in fcs:
                    tiles[fc] = psp.tile([P, NT, 512], F32, tag="big",
                                         bufs=3, name=f"mm1_{fc}")
                for k in range(KO):
                    for fc in fcs:
                        wsl = (waq_t[k][:, fc * P:(fc + 1) * P] if fc < ROQ
                               else wakv_t[k][:, (fc - ROQ) * P:
                                              (fc - ROQ + 1) * P])
                        for nt in range(NT):
                            xsl = (x0h[nt][:, :] if k == 0
                                   else x_t[k][:, nt * 512:(nt + 1) * 512])
                            nc.tensor.matmul(
                                tiles[fc][:, nt, :], wsl, xsl,
                                start=(k == 0), stop=(k == KO - 1))
                for fc in fcs:
                    if fc < ROQ:
                        dst = q_cT[:, fc, :]
                    elif fc < 10:
                        dst = kv_cT[:, fc - ROQ, :]
                    else:
                        dst = rope_mu[:, :]
                    nc.scalar.activation(
                        dst.rearrange("p (nt t) -> p nt t", nt=NT),
                        tiles[fc][:], AF.Identity,
                        bias=biasA_sb[:, fc:fc + 1])
                if fcs == (3, 4, 5):
                    q_sqs = _ln_sq(q_cT, ROQ, "q")
                elif fcs == (10,):
                    _ln(q_cT, q_cT8, ROQ, Q_LR, nbc_q, istq_row, q_sqs, 64)
                    _wkv_loads()
                elif fcs == (9,):
                    kv_sqs = _ln_sq(kv_cT, ROKV, "kv")

            def _krope_block():
                ptile = psp.tile([P, NT, 512], F32, tag="big", bufs=3,
                                 name="ptr")
                for tci in range(TCN):
                    nc.tensor.transpose(
                        ptile[:, tci // 4, (tci % 4) * 64:(tci % 4) * 64 + 64],
                        rope_mu[0:64, tci * P:(tci + 1) * P], ident[:64, :64])
                nc.scalar.copy(
                    krope_t[:].rearrange("p (a b) c -> p a b c", a=2),
                    ptile[:, :, 0:256].rearrange("p a (b c) -> p a b c", c=64))
                tmp = ws.tile([P, TCN, 32], F32, tag="krtmp", bufs=2,
                              name="tmp")
                tmp2 = ws.tile([P, TCN, 32], F32, tag="krtmp", bufs=2,
                               name="tmp2")
                nc.vector.tensor_tensor(tmp[:], krope_t[:, :, 32:64],
                                        sin_sb[:, :, 0:32], OP.mult)
                nc.vector.tensor_tensor(tmp2[:], krope_t[:, :, 0:32],
                                        sin_sb[:, :, 32:64], OP.mult)
                nc.vector.tensor_tensor(krot[:], krope_t[:], cos_sb[:],
                                        OP.mult)
                nc.vector.tensor_tensor(krot[:, :, 0:32], krot[:, :, 0:32],
                                        tmp[:], OP.subtract)
                nc.vector.tensor_tensor(krot[:, :, 32:64], krot[:, :, 32:64],
                                        tmp2[:], OP.add)

            # ================= phase 2: up-projections + assemble ==========
            outT = out_d.rearrange("h t c -> t h c")

            # stores rotate over four engine queues; deferred by a few
            # groups so no engine's stream blocks on a not-yet-ready ob
            # scalar+sync only: gpsimd issues descriptors ~1us apart and its
            # queue drains slower, which stalls ob recycling
            store_engs = [nc.scalar, nc.sync]
            st_state = {"i": 0, "pending": [], "depth": 3}

            def _queue_store(dst, src):
                st_state["pending"].append((dst, src))
                while len(st_state["pending"]) > st_state["depth"]:
                    d, s = st_state["pending"].pop(0)
                    store_engs[st_state["i"] % len(store_engs)].dma_start(
                        d, s[:])
                    st_state["i"] += 1

            def _flush_stores():
                while st_state["pending"]:
                    d, s = st_state["pending"].pop(0)
                    store_engs[st_state["i"] % len(store_engs)].dma_start(
                        d, s[:])
                    st_state["i"] += 1

            def _q_pass(half, tcis=range(TCN)):
                h0 = half * HH
                c0q = half * QW
                for tci in tcis:
                    tsl = slice(tci * P, (tci + 1) * P)
                    for s in range(2):
                        ob = ws.tile([P, 4, QK_HEAD], F32, tag="obq", bufs=4,
                                     name=f"obq_{half}_{tci}_{s}")
                        obv = ob.rearrange("p (i j) c -> p i j c", j=2)
                        psq = psp.tile([P, NT, 512], F32, tag="big", bufs=3,
                                       name=f"psq_{half}_{tci}_{s}")
                        for i in range(2):
                            gi = 2 * s + i
                            csl = slice(c0q + gi * 2 * QK_HEAD,
                                        c0q + (gi + 1) * 2 * QK_HEAD)
                            nc.tensor.matmul(
                                psq[:, i, 0:2 * QK_HEAD],
                                q_cT8[:, :, tsl], wq8[:, :, csl],
                                start=True, stop=False, perf_mode=DR)
                            for r2 in range(ROQ - 2):
                                nc.tensor.matmul(
                                    psq[:, i, 0:2 * QK_HEAD],
                                    q_cT[:, r2 + 2, tsl], wq16[:, r2, csl],
                                    start=False, stop=(r2 == ROQ - 3))
                        src = psq[:, :, 0:2 * QK_HEAD].rearrange(
                            "p i (j c) -> p i j c", c=QK_HEAD)
                        nc.scalar.activation(obv[:], src[:], AF.Identity,
                                             scale=istq_t[:, tci:tci + 1])
                        if bqb_bc is not None:
                            nc.vector.tensor_tensor(
                                ob[:], ob[:],
                                bqb_bc[:, c0q + s * 768:c0q + s * 768 + 768
                                       ].rearrange("p (i c) -> p i c", c=192),
                                OP.add)
                        # rope in place on SBUF, 4 heads at a time
                        orp = ob[:, :, QK_NOPE:QK_HEAD]
                        cosb = cos_sb[:, tci:tci + 1, :].to_broadcast(
                            [P, 4, QK_ROPE])
                        sinb = sin_sb[:, tci:tci + 1, :].to_broadcast(
                            [P, 4, QK_ROPE])
                        t1 = ws.tile([P, 4, 32], F32, tag="t1", bufs=2,
                                     name=f"t1_{half}_{tci}_{s}")
                        t2 = ws.tile([P, 4, 32], F32, tag="t2", bufs=2,
                                     name=f"t2_{half}_{tci}_{s}")
                        nc.vector.tensor_tensor(t1[:], orp[:, :, 32:64],
                                                sinb[:, :, 0:32], OP.mult)
                        nc.vector.tensor_tensor(t2[:], orp[:, :, 0:32],
                                                sinb[:, :, 32:64], OP.mult)
                        nc.vector.tensor_tensor(orp[:], orp[:], cosb[:],
                                                OP.mult)
                        nc.vector.tensor_tensor(orp[:, :, 0:32],
                                                orp[:, :, 0:32], t1[:],
                                                OP.subtract)
                        nc.vector.tensor_tensor(orp[:, :, 32:64],
                                                orp[:, :, 32:64], t2[:],
                                                OP.add)
                        _queue_store(
                            outT[tsl, h0 + 4 * s:h0 + 4 * s + 4, 0:QK_HEAD],
                            ob)

            def _kv_pass(half, tcis=range(TCN)):
                h0 = half * HH
                c0kv = half * KW
                for tci in tcis:
                    tsl = slice(tci * P, (tci + 1) * P)
                    for s in range(2):
                        ob = ws.tile([P, 4, OUT_C - QK_HEAD], F32, tag="obkv",
                                     bufs=4, name=f"obkv_{half}_{tci}_{s}")
                        obv = ob.rearrange("p (i j) c -> p i j c", j=2)
                        pskv = psp.tile([P, NT, 512], F32, tag="big", bufs=3,
                                        name=f"pskv_{half}_{tci}_{s}")
                        for i in range(2):
                            gi = 2 * s + i
                            csl = slice(c0kv + gi * 512, c0kv + (gi + 1) * 512)
                            nc.tensor.matmul(
                                pskv[:, i, :], kv_cT8[:, :, tsl],
                                wkv8[:, :, csl],
                                start=True, stop=False, perf_mode=DR)
                            for r2 in range(ROKV - 2):
                                nc.tensor.matmul(
                                    pskv[:, i, :], kv_cT[:, r2 + 2, tsl],
                                    wkv_t[(half, r2, gi // 2)][
                                        :, (gi % 2) * 512:(gi % 2) * 512 + 512],
                                    start=False, stop=(r2 == ROKV - 3))
                        src = pskv[:].rearrange("p i (j c) -> p i j c", c=256)
                        # k_nope -> local cols 0:128 (global 192:320)
                        nc.scalar.activation(obv[:, :, :, 0:QK_NOPE],
                                             src[:, :, :, 0:QK_NOPE],
                                             AF.Identity,
                                             scale=istkv_t[:, tci:tci + 1])
                        # v -> local cols 192:320 (global 384:512)
                        nc.vector.tensor_scalar_mul(
                            obv[:, :, :, QK_NOPE + QK_ROPE:],
                            src[:, :, :, QK_NOPE:256],
                            istkv_t[:, tci:tci + 1])
                        if bkvb_bc is not None:
                            bsl = bkvb_bc[:, c0kv + s * 1024:
                                          c0kv + s * 1024 + 1024
                                          ].rearrange(
                                "p (i j two c) -> p i j two c",
                                i=2, two=2, c=128)
                            nc.vector.tensor_tensor(
                                obv[:, :, :, 0:QK_NOPE],
                                obv[:, :, :, 0:QK_NOPE],
                                bsl[:, :, :, 0, :], OP.add)
                            nc.vector.tensor_tensor(
                                obv[:, :, :, QK_NOPE + QK_ROPE:],
                                obv[:, :, :, QK_NOPE + QK_ROPE:],
                                bsl[:, :, :, 1, :], OP.add)
                        # k_rot -> local cols 128:192 (global 320:384)
                        nc.vector.tensor_copy(
                            ob[:, :, QK_NOPE:QK_NOPE + QK_ROPE],
                            krot[:, tci:tci + 1, :].to_broadcast(
                                [P, 4, QK_ROPE]))
                        _queue_store(
                            outT[tsl, h0 + 4 * s:h0 + 4 * s + 4,
                                 QK_HEAD:OUT_C], ob)

            # PE order: a few q tcis first so the kv stat matmuls (which wait
            # on vector squares) and kv LN overlap with q up-proj matmuls.
            # Then interleave kv and q tcis so the store stream is flat
            # (~290 GB/s) instead of a kv-heavy burst that outruns DMA.
            # The ist transposes are emitted at PE points where the istd
            # rows are long since computed, so PE never waits on them.
            _ist_tpose(istq_t, istq_row, "q")
            _q_pass(0, range(0, 3))
            _krope_block()
            _ln(kv_cT, kv_cT8, ROKV, KV_LR, nbc_kv, istkv_row, kv_sqs, 96)
            _q_pass(0, range(3, 4))
            _ist_tpose(istkv_t, istkv_row, "kv")
            kv_list = [(0, t) for t in range(TCN)] + [(1, t) for t in range(TCN)]
            q_list = [(0, t) for t in range(4, TCN)] + \
                     [(1, t) for t in range(TCN)]
            for i, (kh, kt) in enumerate(kv_list):
                if i == len(kv_list) - 3:
                    # drain the pending backlog eagerly near the end so the
                    # final stores are in flight before the last matmuls
                    st_state["depth"] = 1
                _kv_pass(kh, tcis=[kt])
                if i >= 4:
                    qh, qt = q_list[i - 4]
                    _q_pass(qh, tcis=[qt])
            _flush_stores()

# ------------------------- host side -------------------------

def _bf16(x):
    return np.ascontiguousarray(x).astype(ml_dtypes.bfloat16)


def _rope_tables(s0):
    pos = np.arange(s0, s0 + T, dtype=np.float64)
    inv = 1.0 / THETA ** (np.arange(0, QK_ROPE, 2, dtype=np.float64) / QK_ROPE)
    fr = pos[:, None] * inv[None, :]
    cos = np.concatenate([np.cos(fr), np.cos(fr)], axis=1).astype(np.float32)
    sin = np.concatenate([np.sin(fr), np.sin(fr)], axis=1).astype(np.float32)
    # [T, 64] -> [P, TCN, 64] with token t = tc*128 + p
    return (cos.reshape(TCN, P, QK_ROPE).transpose(1, 0, 2).copy(),
            sin.reshape(TCN, P, QK_ROPE).transpose(1, 0, 2).copy())


def build_in_maps(inputs):
    f32 = np.float32
    w_qa = np.asarray(inputs["w_qa"], f32)
    w_qb = np.asarray(inputs["w_qb"], f32)
    w_kva = np.asarray(inputs["w_kva"], f32)
    w_kvb = np.asarray(inputs["w_kvb"], f32)
    g_qa_ln = np.asarray(inputs["g_qa_ln"], f32)
    b_qa_ln = np.asarray(inputs["b_qa_ln"], f32)
    g_kva_ln = np.asarray(inputs["g_kva_ln"], f32)
    b_kva_ln = np.asarray(inputs["b_kva_ln"], f32)
    b_qa = np.asarray(inputs["b_qa"], f32)
    b_kva = np.asarray(inputs["b_kva"], f32)
    b_qb = np.asarray(inputs["b_qb"], f32)
    b_kvb = np.asarray(inputs["b_kvb"], f32)

    # combined a-proj weights: q | kv | rope | mu_q | mu_kv | pad
    waT_all = np.zeros((HID, CW), f32)
    waT_all[:, 0:Q_LR] = w_qa.T
    waT_all[:, Q_LR:Q_LR + KV_LR] = w_kva[:KV_LR].T
    waT_all[:, 1280:1344] = w_kva[KV_LR:].T
    waT_all[:, 1280 + 64] = w_qa.sum(axis=0) / Q_LR
    waT_all[:, 1280 + 96] = w_kva[:KV_LR].sum(axis=0) / KV_LR
    waT = _bf16(waT_all)
    # up-proj weights x8 (fold of 1/8 lives in istd); first 256 contraction
    # rows quantize to e4m3 for the DoubleRow matmuls, packed [p, ktile, c]
    wqbTs = (w_qb * g_qa_ln[None, :]).T.astype(f32) * WSC
    wkvbTs = (w_kvb * g_kva_ln[None, :]).T.astype(f32) * WSC
    wqb8 = np.ascontiguousarray(
        wqbTs[:256].reshape(2, P, -1).transpose(1, 0, 2)
    ).astype(ml_dtypes.float8_e4m3)
    wqb16 = _bf16(wqbTs[256:].reshape(ROQ - 2, P, -1).transpose(1, 0, 2))
    wkvb8 = np.ascontiguousarray(
        wkvbTs[:256].reshape(2, P, -1).transpose(1, 0, 2)
    ).astype(ml_dtypes.float8_e4m3)
    wkvb16 = _bf16(wkvbTs[256:].reshape(ROKV - 2, P, -1).transpose(1, 0, 2))
    bqb_eff = (b_qb + w_qb @ b_qa_ln).astype(f32)
    bkvb_eff = (b_kvb + w_kvb @ b_kva_ln).astype(f32)
    biasA = np.zeros((P, NFC), f32)
    biasA[:, 0:ROQ] = b_qa.reshape(ROQ, P).T
    biasA[:, ROQ:10] = b_kva[:KV_LR].reshape(ROKV, P).T
    biasA[0:64, 10] = b_kva[KV_LR:]
    biasA[64, 10] = b_qa.mean()
    biasA[96, 10] = b_kva[:KV_LR].mean()

    has_qb = bool(np.any(bqb_eff))
    has_kvb = bool(np.any(bkvb_eff))

    flat = np.asarray(inputs["hidden_state"], f32).reshape(B * S, HID)
    in_maps = []
    for c in range(N_CORES):
        tok0 = c * T
        cos, sin = _rope_tables(tok0 % S)
        m = {
            "xT": _bf16(flat[tok0:tok0 + T].T),
            "waT": waT, "wqb8": wqb8, "wqb16": wqb16,
            "wkvb8": wkvb8, "wkvb16": wkvb16,
            "biasA": biasA, "cosb": cos, "sinb": sin,
        }
        if has_qb:
            m["bqb"] = bqb_eff
        if has_kvb:
            m["bkvb"] = bkvb_eff
        in_maps.append(m)
    return in_maps, has_qb, has_kvb


_prog_cache = {}


def kernel(hidden_state, w_qa, b_qa, g_qa_ln, b_qa_ln, w_qb, b_qb,
           w_kva, b_kva, g_kva_ln, b_kva_ln, w_kvb, b_kvb):
    inputs = dict(hidden_state=hidden_state, w_qa=w_qa, b_qa=b_qa,
                  g_qa_ln=g_qa_ln, b_qa_ln=b_qa_ln, w_qb=w_qb, b_qb=b_qb,
                  w_kva=w_kva, b_kva=b_kva, g_kva_ln=g_kva_ln,
                  b_kva_ln=b_kva_ln, w_kvb=w_kvb, b_kvb=b_kvb)
    in_maps, has_qb, has_kvb = build_in_maps(inputs)
    key = (has_qb, has_kvb)
    if key not in _prog_cache:
        _prog_cache[key] = _build(1, has_qb, has_kvb)
    nc = _prog_cache[key]

    res = bass2jax.run_bass_via_pjrt(nc, in_maps, n_cores=N_CORES)

    out = np.empty((B, H, S, OUT_C), np.float32)
    for c in range(N_CORES):
        tok0 = c * T
        b = tok0 // S
        s0 = tok0 % S
        out[b, :, s0:s0 + T, :] = res[c]["out"]
    return out



# revision 71
# speedup vs baseline: 1.1746x; 1.1746x over previous
"""MLA q/k/v projection kernel for Trainium2 (8 NeuronCores, token-data-parallel).

Self-contained: hardcodes the problem shapes from nn_MLA_81106162418389.
  hidden_state [2, 4096, 2048] f32 -> out [2, 16, 4096, 512] f32
Strategy: shard the 8192 tokens over 8 cores (1024 each); replicate weights.
All matmul operands in bf16; single persistent pool structure (no phase
barriers); PSUM = 3x[P,2,512] + 2x[1,512] ring (8 banks exactly).
"""
import sys
sys.path.insert(0, "/opt/trn_rl_repo")

import numpy as np
import ml_dtypes

import concourse.bass as bass
import concourse.tile as tile
from concourse import bacc, mybir
from concourse import bass2jax
from concourse.masks import make_identity


# ---- problem constants ----
HID, QK_NOPE, QK_ROPE, Q_LR, KV_LR, H, V_DIM = 2048, 128, 64, 768, 512, 16, 128
QK_HEAD = QK_NOPE + QK_ROPE           # 192
OUT_C = 2 * QK_HEAD + V_DIM           # 512
B, S = 2, 4096
THETA = 10000.0
EPS = 1e-5

N_CORES = 8
T = (B * S) // N_CORES                # 1024 tokens per core
P = 128
TCN = T // P                          # 8 token chunks
NT = 2                                # 512-wide token tiles for phase 1
KO = HID // P                         # 16 k-chunks for a-proj
ROQ = Q_LR // P                       # 6 r-chunks for q up-proj
ROKV = KV_LR // P                     # 4 r-chunks for kv up-proj
HH = H // 2                           # 8 heads per half-pass
QW = HH * QK_HEAD                     # 1536 q cols per half
KW = HH * (QK_NOPE + V_DIM)           # 2048 kv cols per half
# combined a-proj output columns: q 0:768 | kv 768:1280 | rope 1280:1344 |
# mu_q 1344 | mu_kv 1345 | pad to 11 chunks of 128.  The mu columns hold
# rowsum(W)/dim so the LN means come out of the same matmuls for free.
NFC = 11
CW = NFC * P                          # 1408

F32 = mybir.dt.float32
BF16 = mybir.dt.bfloat16
FP8 = mybir.dt.float8e4
DR = mybir.MatmulPerfMode.DoubleRow
AF = mybir.ActivationFunctionType
OP = mybir.AluOpType
# first 256 contraction dims of each up-proj run as fp8 DoubleRow (2x PE
# rate); weights are pre-scaled x8 on the host so w*8 ~ N(0,0.16) clears
# the e4m3 subnormal band, and 1/8 is folded into istd via the Sqrt scale.
WSC = 8.0


def _build(n_repeats=1, has_qb_bias=False, has_kvb_bias=False):
    nc = bacc.Bacc("TRN2", target_bir_lowering=False, debug=False,
                   num_devices=N_CORES)

    xT_d = nc.dram_tensor("xT", [HID, T], BF16, kind="ExternalInput").ap()
    waT_d = nc.dram_tensor("waT", [HID, CW], BF16, kind="ExternalInput").ap()
    wqb8_d = nc.dram_tensor("wqb8", [P, 2, H * QK_HEAD], FP8,
                            kind="ExternalInput").ap()
    wqb16_d = nc.dram_tensor("wqb16", [P, ROQ - 2, H * QK_HEAD], BF16,
                             kind="ExternalInput").ap()
    wkvb8_d = nc.dram_tensor("wkvb8", [P, 2, H * (QK_NOPE + V_DIM)], FP8,
                             kind="ExternalInput").ap()
    wkvb16_d = nc.dram_tensor("wkvb16", [P, ROKV - 2, H * (QK_NOPE + V_DIM)],
                              BF16, kind="ExternalInput").ap()
    biasA_d = nc.dram_tensor("biasA", [P, NFC], F32, kind="ExternalInput").ap()
    cos_d = nc.dram_tensor("cosb", [P, TCN, QK_ROPE], F32,
                           kind="ExternalInput").ap()
    sin_d = nc.dram_tensor("sinb", [P, TCN, QK_ROPE], F32,
                           kind="ExternalInput").ap()
    bqb_d = bkvb_d = None
    if has_qb_bias:
        bqb_d = nc.dram_tensor("bqb", [H * QK_HEAD], F32, kind="ExternalInput").ap()
    if has_kvb_bias:
        bkvb_d = nc.dram_tensor("bkvb", [H * (QK_NOPE + V_DIM)], F32,
                                kind="ExternalInput").ap()
    # outputs in 4-head blocks, token-major: each token's 4-head slab is one
    # contiguous dram run (3-5KB), so stores need 128 descriptors not 512
    outq_d = nc.dram_tensor("outq", [4, T, 4, QK_HEAD], F32,
                            kind="ExternalOutput").ap()
    outkv_d = nc.dram_tensor("outkv", [4, T, 4, OUT_C - QK_HEAD], F32,
                             kind="ExternalOutput").ap()

    for _ in range(n_repeats):
        _emit_once(nc, xT_d, waT_d, wqb8_d, wqb16_d, wkvb8_d, wkvb16_d,
                   biasA_d, cos_d, sin_d, bqb_d, bkvb_d, outq_d, outkv_d)
    nc.compile()
    return nc


def _emit_once(nc, xT_d, waT_d, wqb8_d, wqb16_d, wkvb8_d, wkvb16_d,
               biasA_d, cos_d, sin_d, bqb_d, bkvb_d, outq_d, outkv_d):
    with tile.TileContext(nc) as tc:
        with tc.tile_pool(name="pp", bufs=1) as pp, \
             tc.tile_pool(name="ws", bufs=1) as ws, \
             tc.tile_pool(name="psp", bufs=1, space="PSUM") as psp:

            # ---- persistent smalls (x0/wa0 jump the DMA queue below) ----
            biasA_sb = pp.tile([P, NFC], F32)
            cos_sb = pp.tile([P, TCN, QK_ROPE], F32)
            sin_sb = pp.tile([P, TCN, QK_ROPE], F32)
            bqb_bc = bkvb_bc = None
            if bqb_d is not None:
                b1 = pp.tile([1, H * QK_HEAD], F32)
                nc.sync.dma_start(b1[:], bqb_d[None, :])
                bqb_bc = pp.tile([P, H * QK_HEAD], F32)
                nc.gpsimd.partition_broadcast(bqb_bc[:], b1[:])
            if bkvb_d is not None:
                b2 = pp.tile([1, H * (QK_NOPE + V_DIM)], F32)
                nc.sync.dma_start(b2[:], bkvb_d[None, :])
                bkvb_bc = pp.tile([P, H * (QK_NOPE + V_DIM)], F32)
                nc.gpsimd.partition_broadcast(bkvb_bc[:], b2[:])

            ones_b = pp.tile([P, 1], BF16)
            nc.gpsimd.memset(ones_b[:], 1.0)
            warm = pp.tile([P, P], BF16)
            nc.gpsimd.memset(warm[:], 0.0)
            # Sqrt runs with scale=WSC^2 so istd comes out as 1/(WSC*std),
            # compensating the x WSC pre-scale baked into the up-proj weights
            eps_t = pp.tile([1, 1], F32)
            nc.gpsimd.memset(eps_t[:], WSC * WSC * EPS)
            ident = pp.tile([P, P], F32)
            make_identity(nc, ident[:])

            # ---- persistent activations ----
            q_cT = pp.tile([P, ROQ, T], BF16)
            kv_cT = pp.tile([P, ROKV, T], BF16)
            q_cT8 = pp.tile([P, 2, T], FP8)
            kv_cT8 = pp.tile([P, 2, T], FP8)
            # rows 0:64 = k_rope; row 64 = mu_q; row 96 = mu_kv (single-
            # partition accesses must start at a 32-partition boundary)
            rope_mu = pp.tile([P, T], F32)
            krope_t = pp.tile([P, TCN, QK_ROPE], F32)
            krot = pp.tile([P, TCN, QK_ROPE], F32)
            nbc_q = pp.tile([P, T], BF16)
            nbc_kv = pp.tile([P, T], BF16)
            istq_t = pp.tile([P, TCN], F32)
            istkv_t = pp.tile([P, TCN], F32)
            istq_row = pp.tile([1, T], F32)
            istkv_row = pp.tile([1, T], F32)

            # ---- bulk loads, in consumption order on one queue ----
            # The x/wa stream needs ~320 GB/s for the first ~20us to keep
            # mm1 fed: nothing else may share HBM until it is done.
            # q-columns ride the critical front stream with x; the kv/rope/mu
            # columns (needed ~20us later) load in a second stream so the
            # front stays under the ~400 GB/s HBM ceiling.
            x_t, waq_t, wakv_t = [], [], []
            for k in range(KO):
                xt = ws.tile([P, T], BF16, tag="x", bufs=KO, name=f"x_{k}")
                nc.sync.dma_start(xt[:], xT_d[k * P:(k + 1) * P, :])
                x_t.append(xt)
                wt = ws.tile([P, Q_LR], BF16, tag="waq", bufs=KO,
                             name=f"waq_{k}")
                nc.sync.dma_start(wt[:], waT_d[k * P:(k + 1) * P, 0:Q_LR])
                waq_t.append(wt)
                if k == 0:
                    nc.sync.dma_start(biasA_sb[:], biasA_d[:])
            for k in range(KO):
                wt = ws.tile([P, CW - Q_LR], BF16, tag="wakv", bufs=KO,
                             name=f"wakv_{k}")
                nc.sync.dma_start(wt[:], waT_d[k * P:(k + 1) * P, Q_LR:CW])
                wakv_t.append(wt)
            nc.sync.dma_start(cos_sb[:], cos_d[:])
            nc.sync.dma_start(sin_sb[:], sin_d[:])
            wq8 = ws.tile([P, 2, H * QK_HEAD], FP8, tag="wq8", bufs=1,
                          name="wq8")
            nc.sync.dma_start(wq8[:], wqb8_d)
            wq16 = ws.tile([P, ROQ - 2, H * QK_HEAD], BF16, tag="wq", bufs=1,
                           name="wq16")
            nc.sync.dma_start(wq16[:], wqb16_d)
            wkv8 = ws.tile([P, 2, H * (QK_NOPE + V_DIM)], FP8, tag="wkv8",
                           bufs=1, name="wkv8")
            nc.sync.dma_start(wkv8[:], wkvb8_d)
            # bf16 kv up-proj rows (ro 2,3) ride the freed x slots; their
            # slot-blocked waits live on the idle gpsimd queue so the sync
            # queue is clean for phase-2 stores.  The dma_starts are EMITTED
            # later (after the q-LN broadcasts) so the gpsimd engine stream
            # does not stall the LN mean-broadcasts behind the slot waits.
            wkv_t = {}
            for half in range(2):
                for ro2 in range(ROKV - 2):
                    for piece in range(2):
                        t = ws.tile([P, T], BF16, tag="x", bufs=KO,
                                    name=f"wkv_{half}_{ro2}_{piece}")
                        wkv_t[(half, ro2, piece)] = t

            def _wkv_loads():
                for half in range(2):
                    c0kv = half * KW
                    for ro2 in range(ROKV - 2):
                        for piece in range(2):
                            nc.gpsimd.dma_start(
                                wkv_t[(half, ro2, piece)][:],
                                wkvb16_d[:, ro2,
                                         c0kv + piece * 1024:
                                         c0kv + (piece + 1) * 1024])

            # ---- PE p-state warmup during the DMA lead-in ----
            warm_ps = psp.tile([1, 512], F32, tag="st", bufs=2, name="warm")
            for _ in range(24):
                nc.tensor.matmul(warm_ps[:, 0:P], ones_b[:], warm[:, :],
                                 start=True, stop=True)

            # ================= phase 1: a-projections + LN =================
            def _ln_sq(src, nfc, which):
                # squares summed across r-chunks on DVE, so each LN stat is
                # a single PE matmul instead of nfc accumulating ones
                sqs = {}
                for nt in range(NT):
                    nts = slice(nt * 512, (nt + 1) * 512)
                    acc = ws.tile([P, 512], BF16, tag="ssum", bufs=2,
                                  name=f"ss_{which}_{nt}")
                    nc.vector.tensor_tensor(acc[:], src[:, 0, nts],
                                            src[:, 0, nts], OP.mult)
                    for fc in range(1, nfc):
                        sq = ws.tile([P, 512], BF16, tag="sq", bufs=2,
                                     name=f"sq_{which}_{nt}_{fc}")
                        nc.vector.tensor_tensor(sq[:], src[:, fc, nts],
                                                src[:, fc, nts], OP.mult)
                        nc.vector.tensor_tensor(acc[:], acc[:], sq[:],
                                                OP.add)
                    sqs[nt] = acc
                return sqs

            def _ln(src, src8, nfc, dim, nbc, istd_row, sqs, mu_part):
                # mean-subtract src in place (mu came out of the a-proj's mu
                # column); 1/(WSC*std) goes to istd_row, applied later as a
                # per-partition scale on the phase-2 copies.  The first two
                # r-chunks are written as fp8 for the DoubleRow matmuls.
                # subtracts run FIRST in the DVE stream: the fp8 copies gate
                # the phase-2 DoubleRow matmuls, the istd chain does not
                for nt in range(NT):
                    nts = slice(nt * 512, (nt + 1) * 512)
                    mu = rope_mu[mu_part:mu_part + 1, nts]
                    nh = ws.tile([1, 512], BF16, tag="nh", bufs=2,
                                 name=f"nh_{nt}")
                    nc.vector.tensor_scalar_mul(nh[:], mu, -1.0)
                    nc.gpsimd.partition_broadcast(nbc[:, nts], nh[:])
                for fc in range(nfc):
                    dst = src8[:, fc, :] if fc < 2 else src[:, fc, :]
                    nc.vector.tensor_tensor(dst, src[:, fc, :],
                                            nbc[:], OP.add)
                for nt in range(NT):
                    nts = slice(nt * 512, (nt + 1) * 512)
                    ps_q = psp.tile([1, 512], F32, tag="st", bufs=2,
                                    name=f"psq_{nt}")
                    nc.tensor.matmul(ps_q[:], ones_b[:], sqs[nt][:],
                                     start=True, stop=True)
                    mu = rope_mu[mu_part:mu_part + 1, nts]
                    istd = istd_row[:, nts]
                    nc.vector.tensor_tensor(istd, mu, mu, OP.mult)
                    nc.vector.scalar_tensor_tensor(
                        istd, ps_q[:], 1.0 / dim, istd,
                        OP.mult, OP.subtract)
                    nc.scalar.activation(istd, istd, AF.Sqrt,
                                         bias=eps_t[:, 0:1], scale=WSC * WSC)
                    nc.vector.reciprocal_approx_fast(istd, istd)

            def _ist_tpose(ist_t, istd_row, name):
                # istd [1, T] -> token-major [P, TCN] via PE transposes of
                # [1,128] segments (no DRAM round-trip, no queue blocking).
                # Emitted at a PE point where istd_row is long since ready.
                ptile = psp.tile([P, NT, 512], F32, tag="big", bufs=3,
                                 name=f"istp_{name}")
                for tc in range(TCN):
                    nc.tensor.transpose(ptile[:, 0, tc:tc + 1],
                                        istd_row[:, tc * P:(tc + 1) * P],
                                        ident[:1, :1])
                nc.scalar.copy(ist_t[:], ptile[:, 0, 0:TCN])

            # combined a-proj over 11 fc chunks; mu columns ride chunk 10
            kv_sqs = None
            for fcs in ((0, 1, 2), (3, 4, 5), (10,), (6, 7, 8), (9,)):
                tiles = {}
                for fc in fcs:
                    tiles[fc] = psp.tile([P, NT, 512], F32, tag="big",
                                         bufs=3, name=f"mm1_{fc}")
                for k in range(KO):
                    for fc in fcs:
                        wsl = (waq_t[k][:, fc * P:(fc + 1) * P] if fc < ROQ
                               else wakv_t[k][:, (fc - ROQ) * P:
                                              (fc - ROQ + 1) * P])
                        for nt in range(NT):
                            nc.tensor.matmul(
                                tiles[fc][:, nt, :], wsl,
                                x_t[k][:, nt * 512:(nt + 1) * 512],
                                start=(k == 0), stop=(k == KO - 1))
                for fc in fcs:
                    if fc < ROQ:
                        dst = q_cT[:, fc, :]
                    elif fc < 10:
                        dst = kv_cT[:, fc - ROQ, :]
                    else:
                        dst = rope_mu[:, :]
                    nc.scalar.activation(
                        dst.rearrange("p (nt t) -> p nt t", nt=NT),
                        tiles[fc][:], AF.Identity,
                        bias=biasA_sb[:, fc:fc + 1])
                if fcs == (3, 4, 5):
                    q_sqs = _ln_sq(q_cT, ROQ, "q")
                elif fcs == (10,):
                    _ln(q_cT, q_cT8, ROQ, Q_LR, nbc_q, istq_row, q_sqs, 64)
                    _wkv_loads()
                elif fcs == (9,):
                    kv_sqs = _ln_sq(kv_cT, ROKV, "kv")

            def _krope_block():
                ptile = psp.tile([P, NT, 512], F32, tag="big", bufs=3,
                                 name="ptr")
                for tci in range(TCN):
                    nc.tensor.transpose(
                        ptile[:, tci // 4, (tci % 4) * 64:(tci % 4) * 64 + 64],
                        rope_mu[0:64, tci * P:(tci + 1) * P], ident[:64, :64])
                nc.scalar.copy(
                    krope_t[:].rearrange("p (a b) c -> p a b c", a=2),
                    ptile[:, :, 0:256].rearrange("p a (b c) -> p a b c", c=64))
                tmp = ws.tile([P, TCN, 32], F32, tag="krtmp", bufs=2,
                              name="tmp")
                tmp2 = ws.tile([P, TCN, 32], F32, tag="krtmp", bufs=2,
                               name="tmp2")
                nc.vector.tensor_tensor(tmp[:], krope_t[:, :, 32:64],
                                        sin_sb[:, :, 0:32], OP.mult)
                nc.vector.tensor_tensor(tmp2[:], krope_t[:, :, 0:32],
                                        sin_sb[:, :, 32:64], OP.mult)
                nc.vector.tensor_tensor(krot[:], krope_t[:], cos_sb[:],
                                        OP.mult)
                nc.vector.tensor_tensor(krot[:, :, 0:32], krot[:, :, 0:32],
                                        tmp[:], OP.subtract)
                nc.vector.tensor_tensor(krot[:, :, 32:64], krot[:, :, 32:64],
                                        tmp2[:], OP.add)

            # ================= phase 2: up-projections + assemble ==========

            # stores rotate over four engine queues; deferred by a few
            # groups so no engine's stream blocks on a not-yet-ready ob
            # scalar+sync only: gpsimd issues descriptors ~1us apart and its
            # queue drains slower, which stalls ob recycling
            store_engs = [nc.scalar, nc.sync]
            st_state = {"i": 0, "pending": [], "depth": 3}

            def _queue_store(dst, src):
                st_state["pending"].append((dst, src))
                while len(st_state["pending"]) > st_state["depth"]:
                    d, s = st_state["pending"].pop(0)
                    store_engs[st_state["i"] % len(store_engs)].dma_start(
                        d, s[:])
                    st_state["i"] += 1

            def _flush_stores():
                while st_state["pending"]:
                    d, s = st_state["pending"].pop(0)
                    store_engs[st_state["i"] % len(store_engs)].dma_start(
                        d, s[:])
                    st_state["i"] += 1

            def _q_pass(half, tcis=range(TCN)):
                h0 = half * HH
                c0q = half * QW
                for tci in tcis:
                    tsl = slice(tci * P, (tci + 1) * P)
                    for s in range(2):
                        ob = ws.tile([P, 4, QK_HEAD], F32, tag="obq", bufs=4,
                                     name=f"obq_{half}_{tci}_{s}")
                        obv = ob.rearrange("p (i j) c -> p i j c", j=2)
                        psq = psp.tile([P, NT, 512], F32, tag="big", bufs=3,
                                       name=f"psq_{half}_{tci}_{s}")
                        for i in range(2):
                            gi = 2 * s + i
                            csl = slice(c0q + gi * 2 * QK_HEAD,
                                        c0q + (gi + 1) * 2 * QK_HEAD)
                            nc.tensor.matmul(
                                psq[:, i, 0:2 * QK_HEAD],
                                q_cT8[:, :, tsl], wq8[:, :, csl],
                                start=True, stop=False, perf_mode=DR)
                            for r2 in range(ROQ - 2):
                                nc.tensor.matmul(
                                    psq[:, i, 0:2 * QK_HEAD],
                                    q_cT[:, r2 + 2, tsl], wq16[:, r2, csl],
                                    start=False, stop=(r2 == ROQ - 3))
                        src = psq[:, :, 0:2 * QK_HEAD].rearrange(
                            "p i (j c) -> p i j c", c=QK_HEAD)
                        nc.scalar.activation(obv[:], src[:], AF.Identity,
                                             scale=istq_t[:, tci:tci + 1])
                        if bqb_bc is not None:
                            nc.vector.tensor_tensor(
                                ob[:], ob[:],
                                bqb_bc[:, c0q + s * 768:c0q + s * 768 + 768
                                       ].rearrange("p (i c) -> p i c", c=192),
                                OP.add)
                        # rope in place on SBUF, 4 heads at a time
                        orp = ob[:, :, QK_NOPE:QK_HEAD]
                        cosb = cos_sb[:, tci:tci + 1, :].to_broadcast(
                            [P, 4, QK_ROPE])
                        sinb = sin_sb[:, tci:tci + 1, :].to_broadcast(
                            [P, 4, QK_ROPE])
                        t1 = ws.tile([P, 4, 32], F32, tag="t1", bufs=2,
                                     name=f"t1_{half}_{tci}_{s}")
                        t2 = ws.tile([P, 4, 32], F32, tag="t2", bufs=2,
                                     name=f"t2_{half}_{tci}_{s}")
                        nc.vector.tensor_tensor(t1[:], orp[:, :, 32:64],
                                                sinb[:, :, 0:32], OP.mult)
                        nc.vector.tensor_tensor(t2[:], orp[:, :, 0:32],
                                                sinb[:, :, 32:64], OP.mult)
                        nc.vector.tensor_tensor(orp[:], orp[:], cosb[:],
                                                OP.mult)
                        nc.vector.tensor_tensor(orp[:, :, 0:32],
                                                orp[:, :, 0:32], t1[:],
                                                OP.subtract)
                        nc.vector.tensor_tensor(orp[:, :, 32:64],
                                                orp[:, :, 32:64], t2[:],
                                                OP.add)
                        _queue_store(outq_d[half * 2 + s, tsl, :, :], ob)

            def _kv_pass(half, tcis=range(TCN)):
                h0 = half * HH
                c0kv = half * KW
                for tci in tcis:
                    tsl = slice(tci * P, (tci + 1) * P)
                    for s in range(2):
                        ob = ws.tile([P, 4, OUT_C - QK_HEAD], F32, tag="obkv",
                                     bufs=4, name=f"obkv_{half}_{tci}_{s}")
                        obv = ob.rearrange("p (i j) c -> p i j c", j=2)
                        pskv = psp.tile([P, NT, 512], F32, tag="big", bufs=3,
                                        name=f"pskv_{half}_{tci}_{s}")
                        for i in range(2):
                            gi = 2 * s + i
                            csl = slice(c0kv + gi * 512, c0kv + (gi + 1) * 512)
                            nc.tensor.matmul(
                                pskv[:, i, :], kv_cT8[:, :, tsl],
                                wkv8[:, :, csl],
                                start=True, stop=False, perf_mode=DR)
                            for r2 in range(ROKV - 2):
                                nc.tensor.matmul(
                                    pskv[:, i, :], kv_cT[:, r2 + 2, tsl],
                                    wkv_t[(half, r2, gi // 2)][
                                        :, (gi % 2) * 512:(gi % 2) * 512 + 512],
                                    start=False, stop=(r2 == ROKV - 3))
                        src = pskv[:].rearrange("p i (j c) -> p i j c", c=256)
                        # k_nope -> local cols 0:128 (global 192:320)
                        nc.scalar.activation(obv[:, :, :, 0:QK_NOPE],
                                             src[:, :, :, 0:QK_NOPE],
                                             AF.Identity,
                                             scale=istkv_t[:, tci:tci + 1])
                        # v -> local cols 192:320 (global 384:512)
                        nc.vector.tensor_scalar_mul(
                            obv[:, :, :, QK_NOPE + QK_ROPE:],
                            src[:, :, :, QK_NOPE:256],
                            istkv_t[:, tci:tci + 1])
                        if bkvb_bc is not None:
                            bsl = bkvb_bc[:, c0kv + s * 1024:
                                          c0kv + s * 1024 + 1024
                                          ].rearrange(
                                "p (i j two c) -> p i j two c",
                                i=2, two=2, c=128)
                            nc.vector.tensor_tensor(
                                obv[:, :, :, 0:QK_NOPE],
                                obv[:, :, :, 0:QK_NOPE],
                                bsl[:, :, :, 0, :], OP.add)
                            nc.vector.tensor_tensor(
                                obv[:, :, :, QK_NOPE + QK_ROPE:],
                                obv[:, :, :, QK_NOPE + QK_ROPE:],
                                bsl[:, :, :, 1, :], OP.add)
                        # k_rot -> local cols 128:192 (global 320:384)
                        nc.vector.tensor_copy(
                            ob[:, :, QK_NOPE:QK_NOPE + QK_ROPE],
                            krot[:, tci:tci + 1, :].to_broadcast(
                                [P, 4, QK_ROPE]))
                        _queue_store(outkv_d[half * 2 + s, tsl, :, :], ob)

            # PE order: a few q tcis first so the kv stat matmuls (which wait
            # on vector squares) and kv LN overlap with q up-proj matmuls.
            # Then interleave kv and q tcis so the store stream is flat
            # (~290 GB/s) instead of a kv-heavy burst that outruns DMA.
            # The ist transposes are emitted at PE points where the istd
            # rows are long since computed, so PE never waits on them.
            _ist_tpose(istq_t, istq_row, "q")
            _q_pass(0, range(0, 3))
            _krope_block()
            _ln(kv_cT, kv_cT8, ROKV, KV_LR, nbc_kv, istkv_row, kv_sqs, 96)
            _q_pass(0, range(3, 4))
            _ist_tpose(istkv_t, istkv_row, "kv")
            kv_list = [(0, t) for t in range(TCN)] + [(1, t) for t in range(TCN)]
            q_list = [(0, t) for t in range(4, TCN)] + \
                     [(1, t) for t in range(TCN)]
            for i, (kh, kt) in enumerate(kv_list):
                if i == len(kv_list) - 3:
                    # drain the pending backlog eagerly near the end so the
                    # final stores are in flight before the last matmuls
                    st_state["depth"] = 1
                _kv_pass(kh, tcis=[kt])
                if i >= 4:
                    qh, qt = q_list[i - 4]
                    _q_pass(qh, tcis=[qt])
            _flush_stores()

# ------------------------- host side -------------------------

def _bf16(x):
    return np.ascontiguousarray(x).astype(ml_dtypes.bfloat16)


def _rope_tables(s0):
    pos = np.arange(s0, s0 + T, dtype=np.float64)
    inv = 1.0 / THETA ** (np.arange(0, QK_ROPE, 2, dtype=np.float64) / QK_ROPE)
    fr = pos[:, None] * inv[None, :]
    cos = np.concatenate([np.cos(fr), np.cos(fr)], axis=1).astype(np.float32)
    sin = np.concatenate([np.sin(fr), np.sin(fr)], axis=1).astype(np.float32)
    # [T, 64] -> [P, TCN, 64] with token t = tc*128 + p
    return (cos.reshape(TCN, P, QK_ROPE).transpose(1, 0, 2).copy(),
            sin.reshape(TCN, P, QK_ROPE).transpose(1, 0, 2).copy())


def build_in_maps(inputs):
    f32 = np.float32
    w_qa = np.asarray(inputs["w_qa"], f32)
    w_qb = np.asarray(inputs["w_qb"], f32)
    w_kva = np.asarray(inputs["w_kva"], f32)
    w_kvb = np.asarray(inputs["w_kvb"], f32)
    g_qa_ln = np.asarray(inputs["g_qa_ln"], f32)
    b_qa_ln = np.asarray(inputs["b_qa_ln"], f32)
    g_kva_ln = np.asarray(inputs["g_kva_ln"], f32)
    b_kva_ln = np.asarray(inputs["b_kva_ln"], f32)
    b_qa = np.asarray(inputs["b_qa"], f32)
    b_kva = np.asarray(inputs["b_kva"], f32)
    b_qb = np.asarray(inputs["b_qb"], f32)
    b_kvb = np.asarray(inputs["b_kvb"], f32)

    # combined a-proj weights: q | kv | rope | mu_q | mu_kv | pad
    waT_all = np.zeros((HID, CW), f32)
    waT_all[:, 0:Q_LR] = w_qa.T
    waT_all[:, Q_LR:Q_LR + KV_LR] = w_kva[:KV_LR].T
    waT_all[:, 1280:1344] = w_kva[KV_LR:].T
    waT_all[:, 1280 + 64] = w_qa.sum(axis=0) / Q_LR
    waT_all[:, 1280 + 96] = w_kva[:KV_LR].sum(axis=0) / KV_LR
    waT = _bf16(waT_all)
    # up-proj weights x8 (fold of 1/8 lives in istd); first 256 contraction
    # rows quantize to e4m3 for the DoubleRow matmuls, packed [p, ktile, c]
    wqbTs = (w_qb * g_qa_ln[None, :]).T.astype(f32) * WSC
    wkvbTs = (w_kvb * g_kva_ln[None, :]).T.astype(f32) * WSC
    wqb8 = np.ascontiguousarray(
        wqbTs[:256].reshape(2, P, -1).transpose(1, 0, 2)
    ).astype(ml_dtypes.float8_e4m3)
    wqb16 = _bf16(wqbTs[256:].reshape(ROQ - 2, P, -1).transpose(1, 0, 2))
    wkvb8 = np.ascontiguousarray(
        wkvbTs[:256].reshape(2, P, -1).transpose(1, 0, 2)
    ).astype(ml_dtypes.float8_e4m3)
    wkvb16 = _bf16(wkvbTs[256:].reshape(ROKV - 2, P, -1).transpose(1, 0, 2))
    bqb_eff = (b_qb + w_qb @ b_qa_ln).astype(f32)
    bkvb_eff = (b_kvb + w_kvb @ b_kva_ln).astype(f32)
    biasA = np.zeros((P, NFC), f32)
    biasA[:, 0:ROQ] = b_qa.reshape(ROQ, P).T
    biasA[:, ROQ:10] = b_kva[:KV_LR].reshape(ROKV, P).T
    biasA[0:64, 10] = b_kva[KV_LR:]
    biasA[64, 10] = b_qa.mean()
    biasA[96, 10] = b_kva[:KV_LR].mean()

    has_qb = bool(np.any(bqb_eff))
    has_kvb = bool(np.any(bkvb_eff))

    flat = np.asarray(inputs["hidden_state"], f32).reshape(B * S, HID)
    in_maps = []
    for c in range(N_CORES):
        tok0 = c * T
        cos, sin = _rope_tables(tok0 % S)
        m = {
            "xT": _bf16(flat[tok0:tok0 + T].T),
            "waT": waT, "wqb8": wqb8, "wqb16": wqb16,
            "wkvb8": wkvb8, "wkvb16": wkvb16,
            "biasA": biasA, "cosb": cos, "sinb": sin,
        }
        if has_qb:
            m["bqb"] = bqb_eff
        if has_kvb:
            m["bkvb"] = bkvb_eff
        in_maps.append(m)
    return in_maps, has_qb, has_kvb


_prog_cache = {}


def kernel(hidden_state, w_qa, b_qa, g_qa_ln, b_qa_ln, w_qb, b_qb,
           w_kva, b_kva, g_kva_ln, b_kva_ln, w_kvb, b_kvb):
    inputs = dict(hidden_state=hidden_state, w_qa=w_qa, b_qa=b_qa,
                  g_qa_ln=g_qa_ln, b_qa_ln=b_qa_ln, w_qb=w_qb, b_qb=b_qb,
                  w_kva=w_kva, b_kva=b_kva, g_kva_ln=g_kva_ln,
                  b_kva_ln=b_kva_ln, w_kvb=w_kvb, b_kvb=b_kvb)
    in_maps, has_qb, has_kvb = build_in_maps(inputs)
    key = (has_qb, has_kvb)
    if key not in _prog_cache:
        _prog_cache[key] = _build(1, has_qb, has_kvb)
    nc = _prog_cache[key]

    res = bass2jax.run_bass_via_pjrt(nc, in_maps, n_cores=N_CORES)

    out = np.empty((B, H, S, OUT_C), np.float32)
    for c in range(N_CORES):
        tok0 = c * T
        b = tok0 // S
        s0 = tok0 % S
        oq = np.asarray(res[c]["outq"])       # [4, T, 4, QK_HEAD]
        okv = np.asarray(res[c]["outkv"])     # [4, T, 4, OUT_C-QK_HEAD]
        out[b, :, s0:s0 + T, 0:QK_HEAD] = \
            oq.transpose(0, 2, 1, 3).reshape(H, T, QK_HEAD)
        out[b, :, s0:s0 + T, QK_HEAD:] = \
            okv.transpose(0, 2, 1, 3).reshape(H, T, OUT_C - QK_HEAD)
    return out



# revision 73
# speedup vs baseline: 1.1885x; 1.0118x over previous
"""MLA q/k/v projection kernel for Trainium2 (8 NeuronCores, token-data-parallel).

Self-contained: hardcodes the problem shapes from nn_MLA_81106162418389.
  hidden_state [2, 4096, 2048] f32 -> out [2, 16, 4096, 512] f32
Strategy: shard the 8192 tokens over 8 cores (1024 each); replicate weights.
All matmul operands in bf16; single persistent pool structure (no phase
barriers); PSUM = 3x[P,2,512] + 2x[1,512] ring (8 banks exactly).
"""
import sys
sys.path.insert(0, "/opt/trn_rl_repo")

import numpy as np
import ml_dtypes

import concourse.bass as bass
import concourse.tile as tile
from concourse import bacc, mybir
from concourse import bass2jax
from concourse.masks import make_identity


# ---- problem constants ----
HID, QK_NOPE, QK_ROPE, Q_LR, KV_LR, H, V_DIM = 2048, 128, 64, 768, 512, 16, 128
QK_HEAD = QK_NOPE + QK_ROPE           # 192
OUT_C = 2 * QK_HEAD + V_DIM           # 512
B, S = 2, 4096
THETA = 10000.0
EPS = 1e-5

N_CORES = 8
T = (B * S) // N_CORES                # 1024 tokens per core
P = 128
TCN = T // P                          # 8 token chunks
NT = 2                                # 512-wide token tiles for phase 1
KO = HID // P                         # 16 k-chunks for a-proj
ROQ = Q_LR // P                       # 6 r-chunks for q up-proj
ROKV = KV_LR // P                     # 4 r-chunks for kv up-proj
HH = H // 2                           # 8 heads per half-pass
QW = HH * QK_HEAD                     # 1536 q cols per half
KW = HH * (QK_NOPE + V_DIM)           # 2048 kv cols per half
# combined a-proj output columns: q 0:768 | kv 768:1280 | rope 1280:1344 |
# mu_q 1344 | mu_kv 1345 | pad to 11 chunks of 128.  The mu columns hold
# rowsum(W)/dim so the LN means come out of the same matmuls for free.
NFC = 11
CW = NFC * P                          # 1408

F32 = mybir.dt.float32
BF16 = mybir.dt.bfloat16
FP8 = mybir.dt.float8e4
DR = mybir.MatmulPerfMode.DoubleRow
AF = mybir.ActivationFunctionType
OP = mybir.AluOpType
# first 256 contraction dims of each up-proj run as fp8 DoubleRow (2x PE
# rate); weights are pre-scaled x8 on the host so w*8 ~ N(0,0.16) clears
# the e4m3 subnormal band, and 1/8 is folded into istd via the Sqrt scale.
WSC = 8.0


def _build(n_repeats=1, has_qb_bias=False, has_kvb_bias=False):
    nc = bacc.Bacc("TRN2", target_bir_lowering=False, debug=False,
                   num_devices=N_CORES)

    xT_d = nc.dram_tensor("xT", [HID, T], BF16, kind="ExternalInput").ap()
    waT_d = nc.dram_tensor("waT", [HID, CW], BF16, kind="ExternalInput").ap()
    wqb8_d = nc.dram_tensor("wqb8", [P, 2, H * QK_HEAD], FP8,
                            kind="ExternalInput").ap()
    wqb16_d = nc.dram_tensor("wqb16", [P, ROQ - 2, H * QK_HEAD], BF16,
                             kind="ExternalInput").ap()
    wkvb8_d = nc.dram_tensor("wkvb8", [P, 2, H * (QK_NOPE + V_DIM)], FP8,
                             kind="ExternalInput").ap()
    wkvb16_d = nc.dram_tensor("wkvb16", [P, ROKV - 2, H * (QK_NOPE + V_DIM)],
                              BF16, kind="ExternalInput").ap()
    biasA_d = nc.dram_tensor("biasA", [P, NFC], F32, kind="ExternalInput").ap()
    cos_d = nc.dram_tensor("cosb", [P, TCN, QK_ROPE], F32,
                           kind="ExternalInput").ap()
    sin_d = nc.dram_tensor("sinb", [P, TCN, QK_ROPE], F32,
                           kind="ExternalInput").ap()
    bqb_d = bkvb_d = None
    if has_qb_bias:
        bqb_d = nc.dram_tensor("bqb", [H * QK_HEAD], F32, kind="ExternalInput").ap()
    if has_kvb_bias:
        bkvb_d = nc.dram_tensor("bkvb", [H * (QK_NOPE + V_DIM)], F32,
                                kind="ExternalInput").ap()
    # outputs in 4-head blocks, token-major: each token's 4-head slab is one
    # contiguous dram run (3-5KB), so stores need 128 descriptors not 512
    outq_d = nc.dram_tensor("outq", [4, T, 4, QK_HEAD], F32,
                            kind="ExternalOutput").ap()
    outkv_d = nc.dram_tensor("outkv", [4, T, 4, OUT_C - QK_HEAD], F32,
                             kind="ExternalOutput").ap()

    for _ in range(n_repeats):
        _emit_once(nc, xT_d, waT_d, wqb8_d, wqb16_d, wkvb8_d, wkvb16_d,
                   biasA_d, cos_d, sin_d, bqb_d, bkvb_d, outq_d, outkv_d)
    nc.compile()
    return nc


def _emit_once(nc, xT_d, waT_d, wqb8_d, wqb16_d, wkvb8_d, wkvb16_d,
               biasA_d, cos_d, sin_d, bqb_d, bkvb_d, outq_d, outkv_d):
    with tile.TileContext(nc) as tc:
        with tc.tile_pool(name="pp", bufs=1) as pp, \
             tc.tile_pool(name="ws", bufs=1) as ws, \
             tc.tile_pool(name="psp", bufs=1, space="PSUM") as psp:

            # ---- persistent smalls (x0/wa0 jump the DMA queue below) ----
            biasA_sb = pp.tile([P, NFC], F32)
            cos_sb = pp.tile([P, TCN, QK_ROPE], F32)
            sin_sb = pp.tile([P, TCN, QK_ROPE], F32)
            bqb_bc = bkvb_bc = None
            if bqb_d is not None:
                b1 = pp.tile([1, H * QK_HEAD], F32)
                nc.sync.dma_start(b1[:], bqb_d[None, :])
                bqb_bc = pp.tile([P, H * QK_HEAD], F32)
                nc.gpsimd.partition_broadcast(bqb_bc[:], b1[:])
            if bkvb_d is not None:
                b2 = pp.tile([1, H * (QK_NOPE + V_DIM)], F32)
                nc.sync.dma_start(b2[:], bkvb_d[None, :])
                bkvb_bc = pp.tile([P, H * (QK_NOPE + V_DIM)], F32)
                nc.gpsimd.partition_broadcast(bkvb_bc[:], b2[:])

            ones_b = pp.tile([P, 1], BF16)
            nc.gpsimd.memset(ones_b[:], 1.0)
            warm = pp.tile([P, P], BF16)
            nc.gpsimd.memset(warm[:], 0.0)
            # Sqrt runs with scale=WSC^2 so istd comes out as 1/(WSC*std),
            # compensating the x WSC pre-scale baked into the up-proj weights
            eps_t = pp.tile([1, 1], F32)
            nc.gpsimd.memset(eps_t[:], WSC * WSC * EPS)
            ident = pp.tile([P, P], F32)
            make_identity(nc, ident[:])

            # ---- persistent activations ----
            q_cT = pp.tile([P, ROQ, T], BF16)
            kv_cT = pp.tile([P, ROKV, T], BF16)
            q_cT8 = pp.tile([P, 2, T], FP8)
            kv_cT8 = pp.tile([P, 2, T], FP8)
            # rows 0:64 = k_rope; row 64 = mu_q; row 96 = mu_kv (single-
            # partition accesses must start at a 32-partition boundary)
            rope_mu = pp.tile([P, T], F32)
            krope_t = pp.tile([P, TCN, QK_ROPE], F32)
            krot = pp.tile([P, TCN, QK_ROPE], F32)
            nbc_q = pp.tile([P, T], BF16)
            nbc_kv = pp.tile([P, T], BF16)
            istq_t = pp.tile([P, TCN], F32)
            istkv_t = pp.tile([P, TCN], F32)
            istq_row = pp.tile([1, T], F32)
            istkv_row = pp.tile([1, T], F32)

            # ---- bulk loads, in consumption order on one queue ----
            # The x/wa stream needs ~320 GB/s for the first ~20us to keep
            # mm1 fed: nothing else may share HBM until it is done.
            # q-columns ride the critical front stream with x; the kv/rope/mu
            # columns (needed ~20us later) load in a second stream so the
            # front stays under the ~400 GB/s HBM ceiling.
            x_t, waq_t, wakv_t = [], [], []
            for k in range(KO):
                xt = ws.tile([P, T], BF16, tag="x", bufs=KO, name=f"x_{k}")
                nc.sync.dma_start(xt[:], xT_d[k * P:(k + 1) * P, :])
                x_t.append(xt)
                wt = ws.tile([P, Q_LR], BF16, tag="waq", bufs=KO,
                             name=f"waq_{k}")
                nc.sync.dma_start(wt[:], waT_d[k * P:(k + 1) * P, 0:Q_LR])
                waq_t.append(wt)
                if k == 0:
                    nc.sync.dma_start(biasA_sb[:], biasA_d[:])
            for k in range(KO):
                wt = ws.tile([P, CW - Q_LR], BF16, tag="wakv", bufs=KO,
                             name=f"wakv_{k}")
                nc.sync.dma_start(wt[:], waT_d[k * P:(k + 1) * P, Q_LR:CW])
                wakv_t.append(wt)
            nc.sync.dma_start(cos_sb[:], cos_d[:])
            nc.sync.dma_start(sin_sb[:], sin_d[:])
            wq8 = ws.tile([P, 2, H * QK_HEAD], FP8, tag="wq8", bufs=1,
                          name="wq8")
            nc.sync.dma_start(wq8[:], wqb8_d)
            wq16 = ws.tile([P, ROQ - 2, H * QK_HEAD], BF16, tag="wq", bufs=1,
                           name="wq16")
            nc.sync.dma_start(wq16[:], wqb16_d)
            wkv8 = ws.tile([P, 2, H * (QK_NOPE + V_DIM)], FP8, tag="wkv8",
                           bufs=1, name="wkv8")
            nc.sync.dma_start(wkv8[:], wkvb8_d)
            # bf16 kv up-proj rows (ro 2,3) ride the freed x slots; their
            # slot-blocked waits live on the idle gpsimd queue so the sync
            # queue is clean for phase-2 stores.  The dma_starts are EMITTED
            # later (after the q-LN broadcasts) so the gpsimd engine stream
            # does not stall the LN mean-broadcasts behind the slot waits.
            wkv_t = {}
            for half in range(2):
                for ro2 in range(ROKV - 2):
                    for piece in range(2):
                        t = ws.tile([P, T], BF16, tag="x", bufs=KO,
                                    name=f"wkv_{half}_{ro2}_{piece}")
                        wkv_t[(half, ro2, piece)] = t

            def _wkv_loads():
                for half in range(2):
                    c0kv = half * KW
                    for ro2 in range(ROKV - 2):
                        for piece in range(2):
                            nc.gpsimd.dma_start(
                                wkv_t[(half, ro2, piece)][:],
                                wkvb16_d[:, ro2,
                                         c0kv + piece * 1024:
                                         c0kv + (piece + 1) * 1024])

            # ---- PE p-state warmup during the DMA lead-in ----
            warm_ps = psp.tile([1, 512], F32, tag="st", bufs=2, name="warm")
            for _ in range(24):
                nc.tensor.matmul(warm_ps[:, 0:P], ones_b[:], warm[:, :],
                                 start=True, stop=True)

            # ================= phase 1: a-projections + LN =================
            def _ln_sq(src, nfc, which):
                # squares summed across r-chunks on DVE, so each LN stat is
                # a single PE matmul instead of nfc accumulating ones
                sqs = {}
                for nt in range(NT):
                    nts = slice(nt * 512, (nt + 1) * 512)
                    acc = ws.tile([P, 512], BF16, tag="ssum", bufs=2,
                                  name=f"ss_{which}_{nt}")
                    nc.vector.tensor_tensor(acc[:], src[:, 0, nts],
                                            src[:, 0, nts], OP.mult)
                    for fc in range(1, nfc):
                        sq = ws.tile([P, 512], BF16, tag="sq", bufs=2,
                                     name=f"sq_{which}_{nt}_{fc}")
                        nc.vector.tensor_tensor(sq[:], src[:, fc, nts],
                                                src[:, fc, nts], OP.mult)
                        nc.vector.tensor_tensor(acc[:], acc[:], sq[:],
                                                OP.add)
                    sqs[nt] = acc
                return sqs

            def _ln(src, src8, nfc, dim, nbc, istd_row, sqs, mu_part):
                # mean-subtract src in place (mu came out of the a-proj's mu
                # column); 1/(WSC*std) goes to istd_row, applied later as a
                # per-partition scale on the phase-2 copies.  The first two
                # r-chunks are written as fp8 for the DoubleRow matmuls.
                # subtracts run FIRST in the DVE stream: the fp8 copies gate
                # the phase-2 DoubleRow matmuls, the istd chain does not
                for nt in range(NT):
                    nts = slice(nt * 512, (nt + 1) * 512)
                    mu = rope_mu[mu_part:mu_part + 1, nts]
                    nh = ws.tile([1, 512], BF16, tag="nh", bufs=2,
                                 name=f"nh_{nt}")
                    nc.vector.tensor_scalar_mul(nh[:], mu, -1.0)
                    nc.gpsimd.partition_broadcast(nbc[:, nts], nh[:])
                for fc in range(nfc):
                    dst = src8[:, fc, :] if fc < 2 else src[:, fc, :]
                    nc.vector.tensor_tensor(dst, src[:, fc, :],
                                            nbc[:], OP.add)
                for nt in range(NT):
                    nts = slice(nt * 512, (nt + 1) * 512)
                    ps_q = psp.tile([1, 512], F32, tag="st", bufs=2,
                                    name=f"psq_{nt}")
                    nc.tensor.matmul(ps_q[:], ones_b[:], sqs[nt][:],
                                     start=True, stop=True)
                    mu = rope_mu[mu_part:mu_part + 1, nts]
                    istd = istd_row[:, nts]
                    nc.vector.tensor_tensor(istd, mu, mu, OP.mult)
                    nc.vector.scalar_tensor_tensor(
                        istd, ps_q[:], 1.0 / dim, istd,
                        OP.mult, OP.subtract)
                    nc.scalar.activation(istd, istd, AF.Sqrt,
                                         bias=eps_t[:, 0:1], scale=WSC * WSC)
                    nc.vector.reciprocal_approx_fast(istd, istd)

            def _ist_tpose(ist_t, istd_row, name):
                # istd [1, T] -> token-major [P, TCN] via PE transposes of
                # [1,128] segments (no DRAM round-trip, no queue blocking).
                # Emitted at a PE point where istd_row is long since ready.
                ptile = psp.tile([P, NT, 512], F32, tag="big", bufs=3,
                                 name=f"istp_{name}")
                for tc in range(TCN):
                    nc.tensor.transpose(ptile[:, 0, tc:tc + 1],
                                        istd_row[:, tc * P:(tc + 1) * P],
                                        ident[:1, :1])
                nc.scalar.copy(ist_t[:], ptile[:, 0, 0:TCN])

            # combined a-proj over 11 fc chunks; mu columns ride chunk 10
            kv_sqs = None
            for fcs in ((0, 1, 2), (3, 4, 5), (10,), (6, 7, 8), (9,)):
                tiles = {}
                for fc in fcs:
                    tiles[fc] = psp.tile([P, NT, 512], F32, tag="big",
                                         bufs=3, name=f"mm1_{fc}")
                for k in range(KO):
                    for fc in fcs:
                        wsl = (waq_t[k][:, fc * P:(fc + 1) * P] if fc < ROQ
                               else wakv_t[k][:, (fc - ROQ) * P:
                                              (fc - ROQ + 1) * P])
                        for nt in range(NT):
                            nc.tensor.matmul(
                                tiles[fc][:, nt, :], wsl,
                                x_t[k][:, nt * 512:(nt + 1) * 512],
                                start=(k == 0), stop=(k == KO - 1))
                for fc in fcs:
                    if fc < ROQ:
                        dst = q_cT[:, fc, :]
                    elif fc < 10:
                        dst = kv_cT[:, fc - ROQ, :]
                    else:
                        dst = rope_mu[:, :]
                    nc.scalar.activation(
                        dst.rearrange("p (nt t) -> p nt t", nt=NT),
                        tiles[fc][:], AF.Identity,
                        bias=biasA_sb[:, fc:fc + 1])
                if fcs == (3, 4, 5):
                    q_sqs = _ln_sq(q_cT, ROQ, "q")
                elif fcs == (10,):
                    _ln(q_cT, q_cT8, ROQ, Q_LR, nbc_q, istq_row, q_sqs, 64)
                    _wkv_loads()
                elif fcs == (9,):
                    kv_sqs = _ln_sq(kv_cT, ROKV, "kv")

            def _krope_block():
                ptile = psp.tile([P, NT, 512], F32, tag="big", bufs=3,
                                 name="ptr")
                for tci in range(TCN):
                    nc.tensor.transpose(
                        ptile[:, tci // 4, (tci % 4) * 64:(tci % 4) * 64 + 64],
                        rope_mu[0:64, tci * P:(tci + 1) * P], ident[:64, :64])
                nc.scalar.copy(
                    krope_t[:].rearrange("p (a b) c -> p a b c", a=2),
                    ptile[:, :, 0:256].rearrange("p a (b c) -> p a b c", c=64))
                tmp = ws.tile([P, TCN, 32], F32, tag="krtmp", bufs=2,
                              name="tmp")
                tmp2 = ws.tile([P, TCN, 32], F32, tag="krtmp", bufs=2,
                               name="tmp2")
                nc.vector.tensor_tensor(tmp[:], krope_t[:, :, 32:64],
                                        sin_sb[:, :, 0:32], OP.mult)
                nc.vector.tensor_tensor(tmp2[:], krope_t[:, :, 0:32],
                                        sin_sb[:, :, 32:64], OP.mult)
                nc.vector.tensor_tensor(krot[:], krope_t[:], cos_sb[:],
                                        OP.mult)
                nc.vector.tensor_tensor(krot[:, :, 0:32], krot[:, :, 0:32],
                                        tmp[:], OP.subtract)
                nc.vector.tensor_tensor(krot[:, :, 32:64], krot[:, :, 32:64],
                                        tmp2[:], OP.add)

            # ================= phase 2: up-projections + assemble ==========

            # stores rotate over four engine queues; deferred by a few
            # groups so no engine's stream blocks on a not-yet-ready ob
            # scalar+sync only: gpsimd issues descriptors ~1us apart and its
            # queue drains slower, which stalls ob recycling
            store_engs = [nc.scalar, nc.sync]
            st_state = {"i": 0, "pending": [], "depth": 3}

            def _queue_store(dst, src):
                st_state["pending"].append((dst, src))
                while len(st_state["pending"]) > st_state["depth"]:
                    d, s = st_state["pending"].pop(0)
                    store_engs[st_state["i"] % len(store_engs)].dma_start(
                        d, s[:])
                    st_state["i"] += 1

            def _flush_stores():
                while st_state["pending"]:
                    d, s = st_state["pending"].pop(0)
                    store_engs[st_state["i"] % len(store_engs)].dma_start(
                        d, s[:])
                    st_state["i"] += 1

            def _q_pass(half, tcis=range(TCN)):
                h0 = half * HH
                c0q = half * QW
                for tci in tcis:
                    tsl = slice(tci * P, (tci + 1) * P)
                    for s in range(2):
                        ob = ws.tile([P, 4, QK_HEAD], F32, tag="obq", bufs=4,
                                     name=f"obq_{half}_{tci}_{s}")
                        obv = ob.rearrange("p (i j) c -> p i j c", j=2)
                        psq = psp.tile([P, NT, 512], F32, tag="big", bufs=3,
                                       name=f"psq_{half}_{tci}_{s}")
                        for i in range(2):
                            gi = 2 * s + i
                            csl = slice(c0q + gi * 2 * QK_HEAD,
                                        c0q + (gi + 1) * 2 * QK_HEAD)
                            nc.tensor.matmul(
                                psq[:, i, 0:2 * QK_HEAD],
                                q_cT8[:, :, tsl], wq8[:, :, csl],
                                start=True, stop=False, perf_mode=DR)
                            for r2 in range(ROQ - 2):
                                nc.tensor.matmul(
                                    psq[:, i, 0:2 * QK_HEAD],
                                    q_cT[:, r2 + 2, tsl], wq16[:, r2, csl],
                                    start=False, stop=(r2 == ROQ - 3))
                        src = psq[:, :, 0:2 * QK_HEAD].rearrange(
                            "p i (j c) -> p i j c", c=QK_HEAD)
                        nc.scalar.activation(obv[:], src[:], AF.Identity,
                                             scale=istq_t[:, tci:tci + 1])
                        if bqb_bc is not None:
                            nc.vector.tensor_tensor(
                                ob[:], ob[:],
                                bqb_bc[:, c0q + s * 768:c0q + s * 768 + 768
                                       ].rearrange("p (i c) -> p i c", c=192),
                                OP.add)
                        # rope in place on SBUF, 4 heads at a time
                        orp = ob[:, :, QK_NOPE:QK_HEAD]
                        cosb = cos_sb[:, tci:tci + 1, :].to_broadcast(
                            [P, 4, QK_ROPE])
                        sinb = sin_sb[:, tci:tci + 1, :].to_broadcast(
                            [P, 4, QK_ROPE])
                        t1 = ws.tile([P, 4, 32], F32, tag="t1", bufs=2,
                                     name=f"t1_{half}_{tci}_{s}")
                        t2 = ws.tile([P, 4, 32], F32, tag="t2", bufs=2,
                                     name=f"t2_{half}_{tci}_{s}")
                        nc.vector.tensor_tensor(t1[:], orp[:, :, 32:64],
                                                sinb[:, :, 0:32], OP.mult)
                        nc.vector.tensor_tensor(t2[:], orp[:, :, 0:32],
                                                sinb[:, :, 32:64], OP.mult)
                        nc.vector.tensor_tensor(orp[:], orp[:], cosb[:],
                                                OP.mult)
                        nc.vector.tensor_tensor(orp[:, :, 0:32],
                                                orp[:, :, 0:32], t1[:],
                                                OP.subtract)
                        nc.vector.tensor_tensor(orp[:, :, 32:64],
                                                orp[:, :, 32:64], t2[:],
                                                OP.add)
                        _queue_store(outq_d[half * 2 + s, tsl, :, :], ob)

            def _kv_pass(half, tcis=range(TCN)):
                h0 = half * HH
                c0kv = half * KW
                for tci in tcis:
                    tsl = slice(tci * P, (tci + 1) * P)
                    for s in range(2):
                        ob = ws.tile([P, 4, OUT_C - QK_HEAD], F32, tag="obkv",
                                     bufs=4, name=f"obkv_{half}_{tci}_{s}")
                        obv = ob.rearrange("p (i j) c -> p i j c", j=2)
                        pskv = psp.tile([P, NT, 512], F32, tag="big", bufs=3,
                                        name=f"pskv_{half}_{tci}_{s}")
                        for i in range(2):
                            gi = 2 * s + i
                            csl = slice(c0kv + gi * 512, c0kv + (gi + 1) * 512)
                            nc.tensor.matmul(
                                pskv[:, i, :], kv_cT8[:, :, tsl],
                                wkv8[:, :, csl],
                                start=True, stop=False, perf_mode=DR)
                            for r2 in range(ROKV - 2):
                                nc.tensor.matmul(
                                    pskv[:, i, :], kv_cT[:, r2 + 2, tsl],
                                    wkv_t[(half, r2, gi // 2)][
                                        :, (gi % 2) * 512:(gi % 2) * 512 + 512],
                                    start=False, stop=(r2 == ROKV - 3))
                        src = pskv[:].rearrange("p i (j c) -> p i j c", c=256)
                        # k_nope -> local cols 0:128 (global 192:320)
                        nc.scalar.activation(obv[:, :, :, 0:QK_NOPE],
                                             src[:, :, :, 0:QK_NOPE],
                                             AF.Identity,
                                             scale=istkv_t[:, tci:tci + 1])
                        # v -> local cols 192:320 (global 384:512)
                        nc.vector.tensor_scalar_mul(
                            obv[:, :, :, QK_NOPE + QK_ROPE:],
                            src[:, :, :, QK_NOPE:256],
                            istkv_t[:, tci:tci + 1])
                        if bkvb_bc is not None:
                            bsl = bkvb_bc[:, c0kv + s * 1024:
                                          c0kv + s * 1024 + 1024
                                          ].rearrange(
                                "p (i j two c) -> p i j two c",
                                i=2, two=2, c=128)
                            nc.vector.tensor_tensor(
                                obv[:, :, :, 0:QK_NOPE],
                                obv[:, :, :, 0:QK_NOPE],
                                bsl[:, :, :, 0, :], OP.add)
                            nc.vector.tensor_tensor(
                                obv[:, :, :, QK_NOPE + QK_ROPE:],
                                obv[:, :, :, QK_NOPE + QK_ROPE:],
                                bsl[:, :, :, 1, :], OP.add)
                        # k_rot -> local cols 128:192 (global 320:384)
                        nc.vector.tensor_copy(
                            ob[:, :, QK_NOPE:QK_NOPE + QK_ROPE],
                            krot[:, tci:tci + 1, :].to_broadcast(
                                [P, 4, QK_ROPE]))
                        _queue_store(outkv_d[half * 2 + s, tsl, :, :], ob)

            # PE order: a few q tcis first so the kv stat matmuls (which wait
            # on vector squares) and kv LN overlap with q up-proj matmuls.
            # Then interleave kv and q tcis so the store stream is flat
            # (~290 GB/s) instead of a kv-heavy burst that outruns DMA.
            # The ist transposes are emitted at PE points where the istd
            # rows are long since computed, so PE never waits on them.
            _ist_tpose(istq_t, istq_row, "q")
            _q_pass(0, range(0, 2))
            # lnkv here: its PE stats are covered by the two q passes above
            # (kv squares on DVE), and its istd chain finishes well before
            # the kv ist transposes below
            _ln(kv_cT, kv_cT8, ROKV, KV_LR, nbc_kv, istkv_row, kv_sqs, 96)
            _q_pass(0, range(2, 3))
            _krope_block()
            _q_pass(0, range(3, 4))
            _ist_tpose(istkv_t, istkv_row, "kv")
            # strict kv/q alternation (the 4 surplus kv passes spread evenly)
            # so the store stream never bursts ahead of the two queues
            kv_list = [(0, t) for t in range(TCN)] + [(1, t) for t in range(TCN)]
            q_list = [(0, t) for t in range(4, TCN)] + \
                     [(1, t) for t in range(TCN)]
            ki = 0
            for i, (qh, qt) in enumerate(q_list):
                if i == len(q_list) - 2:
                    # drain the pending backlog eagerly near the end so the
                    # final stores are in flight before the last matmuls
                    st_state["depth"] = 1
                kh, kt = kv_list[ki]; ki += 1
                _kv_pass(kh, tcis=[kt])
                if i in (2, 5, 8, 11):
                    kh, kt = kv_list[ki]; ki += 1
                    _kv_pass(kh, tcis=[kt])
                _q_pass(qh, tcis=[qt])
            _flush_stores()

# ------------------------- host side -------------------------

def _bf16(x):
    return np.ascontiguousarray(x).astype(ml_dtypes.bfloat16)


def _rope_tables(s0):
    pos = np.arange(s0, s0 + T, dtype=np.float64)
    inv = 1.0 / THETA ** (np.arange(0, QK_ROPE, 2, dtype=np.float64) / QK_ROPE)
    fr = pos[:, None] * inv[None, :]
    cos = np.concatenate([np.cos(fr), np.cos(fr)], axis=1).astype(np.float32)
    sin = np.concatenate([np.sin(fr), np.sin(fr)], axis=1).astype(np.float32)
    # [T, 64] -> [P, TCN, 64] with token t = tc*128 + p
    return (cos.reshape(TCN, P, QK_ROPE).transpose(1, 0, 2).copy(),
            sin.reshape(TCN, P, QK_ROPE).transpose(1, 0, 2).copy())


def build_in_maps(inputs):
    f32 = np.float32
    w_qa = np.asarray(inputs["w_qa"], f32)
    w_qb = np.asarray(inputs["w_qb"], f32)
    w_kva = np.asarray(inputs["w_kva"], f32)
    w_kvb = np.asarray(inputs["w_kvb"], f32)
    g_qa_ln = np.asarray(inputs["g_qa_ln"], f32)
    b_qa_ln = np.asarray(inputs["b_qa_ln"], f32)
    g_kva_ln = np.asarray(inputs["g_kva_ln"], f32)
    b_kva_ln = np.asarray(inputs["b_kva_ln"], f32)
    b_qa = np.asarray(inputs["b_qa"], f32)
    b_kva = np.asarray(inputs["b_kva"], f32)
    b_qb = np.asarray(inputs["b_qb"], f32)
    b_kvb = np.asarray(inputs["b_kvb"], f32)

    # combined a-proj weights: q | kv | rope | mu_q | mu_kv | pad
    waT_all = np.zeros((HID, CW), f32)
    waT_all[:, 0:Q_LR] = w_qa.T
    waT_all[:, Q_LR:Q_LR + KV_LR] = w_kva[:KV_LR].T
    waT_all[:, 1280:1344] = w_kva[KV_LR:].T
    waT_all[:, 1280 + 64] = w_qa.sum(axis=0) / Q_LR
    waT_all[:, 1280 + 96] = w_kva[:KV_LR].sum(axis=0) / KV_LR
    waT = _bf16(waT_all)
    # up-proj weights x8 (fold of 1/8 lives in istd); first 256 contraction
    # rows quantize to e4m3 for the DoubleRow matmuls, packed [p, ktile, c]
    wqbTs = (w_qb * g_qa_ln[None, :]).T.astype(f32) * WSC
    wkvbTs = (w_kvb * g_kva_ln[None, :]).T.astype(f32) * WSC
    wqb8 = np.ascontiguousarray(
        wqbTs[:256].reshape(2, P, -1).transpose(1, 0, 2)
    ).astype(ml_dtypes.float8_e4m3)
    wqb16 = _bf16(wqbTs[256:].reshape(ROQ - 2, P, -1).transpose(1, 0, 2))
    wkvb8 = np.ascontiguousarray(
        wkvbTs[:256].reshape(2, P, -1).transpose(1, 0, 2)
    ).astype(ml_dtypes.float8_e4m3)
    wkvb16 = _bf16(wkvbTs[256:].reshape(ROKV - 2, P, -1).transpose(1, 0, 2))
    bqb_eff = (b_qb + w_qb @ b_qa_ln).astype(f32)
    bkvb_eff = (b_kvb + w_kvb @ b_kva_ln).astype(f32)
    biasA = np.zeros((P, NFC), f32)
    biasA[:, 0:ROQ] = b_qa.reshape(ROQ, P).T
    biasA[:, ROQ:10] = b_kva[:KV_LR].reshape(ROKV, P).T
    biasA[0:64, 10] = b_kva[KV_LR:]
    biasA[64, 10] = b_qa.mean()
    biasA[96, 10] = b_kva[:KV_LR].mean()

    has_qb = bool(np.any(bqb_eff))
    has_kvb = bool(np.any(bkvb_eff))

    flat = np.asarray(inputs["hidden_state"], f32).reshape(B * S, HID)
    in_maps = []
    for c in range(N_CORES):
        tok0 = c * T
        cos, sin = _rope_tables(tok0 % S)
        m = {
            "xT": _bf16(flat[tok0:tok0 + T].T),
            "waT": waT, "wqb8": wqb8, "wqb16": wqb16,
            "wkvb8": wkvb8, "wkvb16": wkvb16,
            "biasA": biasA, "cosb": cos, "sinb": sin,
        }
        if has_qb:
            m["bqb"] = bqb_eff
        if has_kvb:
            m["bkvb"] = bkvb_eff
        in_maps.append(m)
    return in_maps, has_qb, has_kvb


_prog_cache = {}


def kernel(hidden_state, w_qa, b_qa, g_qa_ln, b_qa_ln, w_qb, b_qb,
           w_kva, b_kva, g_kva_ln, b_kva_ln, w_kvb, b_kvb):
    inputs = dict(hidden_state=hidden_state, w_qa=w_qa, b_qa=b_qa,
                  g_qa_ln=g_qa_ln, b_qa_ln=b_qa_ln, w_qb=w_qb, b_qb=b_qb,
                  w_kva=w_kva, b_kva=b_kva, g_kva_ln=g_kva_ln,
                  b_kva_ln=b_kva_ln, w_kvb=w_kvb, b_kvb=b_kvb)
    in_maps, has_qb, has_kvb = build_in_maps(inputs)
    key = (has_qb, has_kvb)
    if key not in _prog_cache:
        _prog_cache[key] = _build(1, has_qb, has_kvb)
    nc = _prog_cache[key]

    res = bass2jax.run_bass_via_pjrt(nc, in_maps, n_cores=N_CORES)

    out = np.empty((B, H, S, OUT_C), np.float32)
    for c in range(N_CORES):
        tok0 = c * T
        b = tok0 // S
        s0 = tok0 % S
        oq = np.asarray(res[c]["outq"])       # [4, T, 4, QK_HEAD]
        okv = np.asarray(res[c]["outkv"])     # [4, T, 4, OUT_C-QK_HEAD]
        out[b, :, s0:s0 + T, 0:QK_HEAD] = \
            oq.transpose(0, 2, 1, 3).reshape(H, T, QK_HEAD)
        out[b, :, s0:s0 + T, QK_HEAD:] = \
            okv.transpose(0, 2, 1, 3).reshape(H, T, OUT_C - QK_HEAD)
    return out



# revision 77
# speedup vs baseline: 1.2157x; 1.0229x over previous
"""MLA q/k/v projection kernel for Trainium2 (8 NeuronCores, token-data-parallel).

Self-contained: hardcodes the problem shapes from nn_MLA_81106162418389.
  hidden_state [2, 4096, 2048] f32 -> out [2, 16, 4096, 512] f32
Strategy: shard the 8192 tokens over 8 cores (1024 each); replicate weights.
All matmul operands in bf16; single persistent pool structure (no phase
barriers); PSUM = 3x[P,2,512] + 2x[1,512] ring (8 banks exactly).
"""
import sys
sys.path.insert(0, "/opt/trn_rl_repo")

import numpy as np
import ml_dtypes

import concourse.bass as bass
import concourse.tile as tile
from concourse import bacc, mybir
from concourse import bass2jax
from concourse.masks import make_identity


# ---- problem constants ----
HID, QK_NOPE, QK_ROPE, Q_LR, KV_LR, H, V_DIM = 2048, 128, 64, 768, 512, 16, 128
QK_HEAD = QK_NOPE + QK_ROPE           # 192
OUT_C = 2 * QK_HEAD + V_DIM           # 512
B, S = 2, 4096
THETA = 10000.0
EPS = 1e-5

N_CORES = 8
T = (B * S) // N_CORES                # 1024 tokens per core
P = 128
TCN = T // P                          # 8 token chunks
NT = 2                                # 512-wide token tiles for phase 1
KO = HID // P                         # 16 k-chunks for a-proj
ROQ = Q_LR // P                       # 6 r-chunks for q up-proj
ROKV = KV_LR // P                     # 4 r-chunks for kv up-proj
HH = H // 2                           # 8 heads per half-pass
QW = HH * QK_HEAD                     # 1536 q cols per half
KW = HH * (QK_NOPE + V_DIM)           # 2048 kv cols per half
# combined a-proj output columns: q 0:768 | kv 768:1280 | rope 1280:1344 |
# mu_q 1344 | mu_kv 1345 | pad to 11 chunks of 128.  The mu columns hold
# rowsum(W)/dim so the LN means come out of the same matmuls for free.
NFC = 11
CW = NFC * P                          # 1408

F32 = mybir.dt.float32
BF16 = mybir.dt.bfloat16
FP8 = mybir.dt.float8e4
DR = mybir.MatmulPerfMode.DoubleRow
AF = mybir.ActivationFunctionType
OP = mybir.AluOpType
# first 256 contraction dims of each up-proj run as fp8 DoubleRow (2x PE
# rate); weights are pre-scaled x8 on the host so w*8 ~ N(0,0.16) clears
# the e4m3 subnormal band, and 1/8 is folded into istd via the Sqrt scale.
WSC = 8.0


def _build(n_repeats=1, has_qb_bias=False, has_kvb_bias=False):
    nc = bacc.Bacc("TRN2", target_bir_lowering=False, debug=False,
                   num_devices=N_CORES)

    xT_d = nc.dram_tensor("xT", [HID, T], BF16, kind="ExternalInput").ap()
    waT_d = nc.dram_tensor("waT", [HID, CW], BF16, kind="ExternalInput").ap()
    wqb8_d = nc.dram_tensor("wqb8", [P, 2, H * QK_HEAD], FP8,
                            kind="ExternalInput").ap()
    wqb16_d = nc.dram_tensor("wqb16", [P, ROQ - 2, H * QK_HEAD], BF16,
                             kind="ExternalInput").ap()
    wkvb8_d = nc.dram_tensor("wkvb8", [P, 2, H * (QK_NOPE + V_DIM)], FP8,
                             kind="ExternalInput").ap()
    wkvb16_d = nc.dram_tensor("wkvb16", [P, ROKV - 2, H * (QK_NOPE + V_DIM)],
                              BF16, kind="ExternalInput").ap()
    biasA_d = nc.dram_tensor("biasA", [P, NFC], F32, kind="ExternalInput").ap()
    cos_d = nc.dram_tensor("cosb", [P, TCN, QK_ROPE], F32,
                           kind="ExternalInput").ap()
    sin_d = nc.dram_tensor("sinb", [P, TCN, QK_ROPE], F32,
                           kind="ExternalInput").ap()
    bqb_d = bkvb_d = None
    if has_qb_bias:
        bqb_d = nc.dram_tensor("bqb", [H * QK_HEAD], F32, kind="ExternalInput").ap()
    if has_kvb_bias:
        bkvb_d = nc.dram_tensor("bkvb", [H * (QK_NOPE + V_DIM)], F32,
                                kind="ExternalInput").ap()
    # outputs in 4-head blocks, token-major: each token's 4-head slab is one
    # contiguous dram run (3-5KB), so stores need 128 descriptors not 512
    outq_d = nc.dram_tensor("outq", [4, T, 4, QK_HEAD], F32,
                            kind="ExternalOutput").ap()
    outkv_d = nc.dram_tensor("outkv", [4, T, 4, OUT_C - QK_HEAD], F32,
                             kind="ExternalOutput").ap()

    for _ in range(n_repeats):
        _emit_once(nc, xT_d, waT_d, wqb8_d, wqb16_d, wkvb8_d, wkvb16_d,
                   biasA_d, cos_d, sin_d, bqb_d, bkvb_d, outq_d, outkv_d)
    nc.compile()
    return nc


def _emit_once(nc, xT_d, waT_d, wqb8_d, wqb16_d, wkvb8_d, wkvb16_d,
               biasA_d, cos_d, sin_d, bqb_d, bkvb_d, outq_d, outkv_d):
    with tile.TileContext(nc) as tc:
        with tc.tile_pool(name="pp", bufs=1) as pp, \
             tc.tile_pool(name="ws", bufs=1) as ws, \
             tc.tile_pool(name="psp", bufs=1, space="PSUM") as psp:

            # ---- persistent smalls (x0/wa0 jump the DMA queue below) ----
            biasA_sb = pp.tile([P, NFC], F32)
            cos_sb = pp.tile([P, TCN, QK_ROPE], F32)
            sin_sb = pp.tile([P, TCN, QK_ROPE], F32)
            bqb_bc = bkvb_bc = None
            if bqb_d is not None:
                b1 = pp.tile([1, H * QK_HEAD], F32)
                nc.sync.dma_start(b1[:], bqb_d[None, :])
                bqb_bc = pp.tile([P, H * QK_HEAD], F32)
                nc.gpsimd.partition_broadcast(bqb_bc[:], b1[:])
            if bkvb_d is not None:
                b2 = pp.tile([1, H * (QK_NOPE + V_DIM)], F32)
                nc.sync.dma_start(b2[:], bkvb_d[None, :])
                bkvb_bc = pp.tile([P, H * (QK_NOPE + V_DIM)], F32)
                nc.gpsimd.partition_broadcast(bkvb_bc[:], b2[:])

            ones_b = pp.tile([P, 1], BF16)
            nc.gpsimd.memset(ones_b[:], 1.0)
            warm = pp.tile([P, P], BF16)
            nc.gpsimd.memset(warm[:], 0.0)
            # Sqrt runs with scale=WSC^2 so istd comes out as 1/(WSC*std),
            # compensating the x WSC pre-scale baked into the up-proj weights
            eps_t = pp.tile([1, 1], F32)
            nc.gpsimd.memset(eps_t[:], WSC * WSC * EPS)
            ident = pp.tile([P, P], F32)
            make_identity(nc, ident[:])

            # ---- persistent activations ----
            q_cT = pp.tile([P, ROQ, T], BF16)
            kv_cT = pp.tile([P, ROKV, T], BF16)
            q_cT8 = pp.tile([P, 2, T], FP8)
            kv_cT8 = pp.tile([P, 2, T], FP8)
            # rows 0:64 = k_rope; row 64 = mu_q; row 96 = mu_kv (single-
            # partition accesses must start at a 32-partition boundary)
            rope_mu = pp.tile([P, T], F32)
            krope_t = pp.tile([P, TCN, QK_ROPE], F32)
            krot = pp.tile([P, TCN, QK_ROPE], F32)
            nbc_q = pp.tile([P, T], BF16)
            nbc_kv = pp.tile([P, T], BF16)
            istq_t = pp.tile([P, TCN], F32)
            istkv_t = pp.tile([P, TCN], F32)
            istq_row = pp.tile([1, T], F32)
            istkv_row = pp.tile([1, T], F32)

            # ---- bulk loads, in consumption order on one queue ----
            # The x/wa stream needs ~320 GB/s for the first ~20us to keep
            # mm1 fed: nothing else may share HBM until it is done.
            # q-columns ride the critical front stream with x; the kv/rope/mu
            # columns (needed ~20us later) load in a second stream so the
            # front stays under the ~400 GB/s HBM ceiling.
            x_t, waq_t, wakv_t = [], [], []
            for k in range(KO):
                xt = ws.tile([P, T], BF16, tag="x", bufs=KO, name=f"x_{k}")
                nc.sync.dma_start(xt[:], xT_d[k * P:(k + 1) * P, :])
                x_t.append(xt)
                wt = ws.tile([P, Q_LR], BF16, tag="waq", bufs=KO,
                             name=f"waq_{k}")
                # waq_0 on the (empty) scalar queue, parallel with x_0, so
                # the first matmul's operands both land ~0.5us sooner
                (nc.scalar if k == 0 else nc.sync).dma_start(
                    wt[:], waT_d[k * P:(k + 1) * P, 0:Q_LR])
                waq_t.append(wt)
                if k == 0:
                    nc.scalar.dma_start(biasA_sb[:], biasA_d[:])
            for k in range(KO):
                wt = ws.tile([P, CW - Q_LR], BF16, tag="wakv", bufs=KO,
                             name=f"wakv_{k}")
                nc.sync.dma_start(wt[:], waT_d[k * P:(k + 1) * P, Q_LR:CW])
                wakv_t.append(wt)
            nc.sync.dma_start(cos_sb[:], cos_d[:])
            nc.sync.dma_start(sin_sb[:], sin_d[:])
            wq8 = ws.tile([P, 2, H * QK_HEAD], FP8, tag="wq8", bufs=1,
                          name="wq8")
            nc.sync.dma_start(wq8[:], wqb8_d)
            wq16 = ws.tile([P, ROQ - 2, H * QK_HEAD], BF16, tag="wq", bufs=1,
                           name="wq16")
            nc.sync.dma_start(wq16[:], wqb16_d)
            wkv8 = ws.tile([P, 2, H * (QK_NOPE + V_DIM)], FP8, tag="wkv8",
                           bufs=1, name="wkv8")
            nc.sync.dma_start(wkv8[:], wkvb8_d)
            # bf16 kv up-proj rows (ro 2,3) ride the freed x slots; their
            # slot-blocked waits live on the idle gpsimd queue so the sync
            # queue is clean for phase-2 stores.  The dma_starts are EMITTED
            # later (after the q-LN broadcasts) so the gpsimd engine stream
            # does not stall the LN mean-broadcasts behind the slot waits.
            wkv_t = {}
            for half in range(2):
                for ro2 in range(ROKV - 2):
                    for piece in range(2):
                        t = ws.tile([P, T], BF16, tag="x", bufs=KO,
                                    name=f"wkv_{half}_{ro2}_{piece}")
                        wkv_t[(half, ro2, piece)] = t

            def _wkv_loads():
                for half in range(2):
                    c0kv = half * KW
                    for ro2 in range(ROKV - 2):
                        for piece in range(2):
                            nc.gpsimd.dma_start(
                                wkv_t[(half, ro2, piece)][:],
                                wkvb16_d[:, ro2,
                                         c0kv + piece * 1024:
                                         c0kv + (piece + 1) * 1024])

            # ---- PE p-state warmup during the DMA lead-in ----
            warm_ps = psp.tile([1, 512], F32, tag="st", bufs=2, name="warm")
            for _ in range(24):
                nc.tensor.matmul(warm_ps[:, 0:P], ones_b[:], warm[:, :],
                                 start=True, stop=True)

            # ================= phase 1: a-projections + LN =================
            def _ln_sq(src, nfc, which):
                # squares summed across r-chunks on DVE, so each LN stat is
                # a single PE matmul instead of nfc accumulating ones
                sqs = {}
                for nt in range(NT):
                    nts = slice(nt * 512, (nt + 1) * 512)
                    acc = ws.tile([P, 512], BF16, tag="ssum", bufs=2,
                                  name=f"ss_{which}_{nt}")
                    nc.vector.tensor_tensor(acc[:], src[:, 0, nts],
                                            src[:, 0, nts], OP.mult)
                    for fc in range(1, nfc):
                        sq = ws.tile([P, 512], BF16, tag="sq", bufs=2,
                                     name=f"sq_{which}_{nt}_{fc}")
                        nc.vector.tensor_tensor(sq[:], src[:, fc, nts],
                                                src[:, fc, nts], OP.mult)
                        nc.vector.tensor_tensor(acc[:], acc[:], sq[:],
                                                OP.add)
                    sqs[nt] = acc
                return sqs

            def _ln(src, src8, nfc, dim, nbc, istd_row, sqs, mu_part):
                # mean-subtract src in place (mu came out of the a-proj's mu
                # column); 1/(WSC*std) goes to istd_row, applied later as a
                # per-partition scale on the phase-2 copies.  The first two
                # r-chunks are written as fp8 for the DoubleRow matmuls.
                # subtracts run FIRST in the DVE stream: the fp8 copies gate
                # the phase-2 DoubleRow matmuls, the istd chain does not
                for nt in range(NT):
                    nts = slice(nt * 512, (nt + 1) * 512)
                    mu = rope_mu[mu_part:mu_part + 1, nts]
                    nh = ws.tile([1, 512], BF16, tag="nh", bufs=2,
                                 name=f"nh_{nt}")
                    nc.vector.tensor_scalar_mul(nh[:], mu, -1.0)
                    nc.gpsimd.partition_broadcast(nbc[:, nts], nh[:])
                for fc in range(nfc):
                    dst = src8[:, fc, :] if fc < 2 else src[:, fc, :]
                    nc.vector.tensor_tensor(dst, src[:, fc, :],
                                            nbc[:], OP.add)
                for nt in range(NT):
                    nts = slice(nt * 512, (nt + 1) * 512)
                    ps_q = psp.tile([1, 512], F32, tag="st", bufs=2,
                                    name=f"psq_{nt}")
                    nc.tensor.matmul(ps_q[:], ones_b[:], sqs[nt][:],
                                     start=True, stop=True)
                    mu = rope_mu[mu_part:mu_part + 1, nts]
                    istd = istd_row[:, nts]
                    nc.vector.tensor_tensor(istd, mu, mu, OP.mult)
                    nc.vector.scalar_tensor_tensor(
                        istd, ps_q[:], 1.0 / dim, istd,
                        OP.mult, OP.subtract)
                    nc.scalar.activation(istd, istd, AF.Sqrt,
                                         bias=eps_t[:, 0:1], scale=WSC * WSC)
                    nc.vector.reciprocal_approx_fast(istd, istd)

            def _ist_tpose(ist_t, istd_row, name):
                # istd [1, T] -> token-major [P, TCN] via PE transposes of
                # [1,128] segments (no DRAM round-trip, no queue blocking).
                # Emitted at a PE point where istd_row is long since ready.
                ptile = psp.tile([P, NT, 512], F32, tag="big", bufs=3,
                                 name=f"istp_{name}")
                for tc in range(TCN):
                    nc.tensor.transpose(ptile[:, 0, tc:tc + 1],
                                        istd_row[:, tc * P:(tc + 1) * P],
                                        ident[:1, :1])
                nc.scalar.copy(ist_t[:], ptile[:, 0, 0:TCN])

            # combined a-proj over 11 fc chunks; mu columns ride chunk 10
            kv_sqs = None
            for fcs in ((0, 1, 2), (3, 4, 5), (10,), (6, 7, 8), (9,)):
                tiles = {}
                for fc in fcs:
                    tiles[fc] = psp.tile([P, NT, 512], F32, tag="big",
                                         bufs=3, name=f"mm1_{fc}")
                for k in range(KO):
                    for fc in fcs:
                        wsl = (waq_t[k][:, fc * P:(fc + 1) * P] if fc < ROQ
                               else wakv_t[k][:, (fc - ROQ) * P:
                                              (fc - ROQ + 1) * P])
                        for nt in range(NT):
                            nc.tensor.matmul(
                                tiles[fc][:, nt, :], wsl,
                                x_t[k][:, nt * 512:(nt + 1) * 512],
                                start=(k == 0), stop=(k == KO - 1))
                for fc in fcs:
                    if fc < ROQ:
                        dst = q_cT[:, fc, :]
                    elif fc < 10:
                        dst = kv_cT[:, fc - ROQ, :]
                    else:
                        dst = rope_mu[:, :]
                    nc.scalar.activation(
                        dst.rearrange("p (nt t) -> p nt t", nt=NT),
                        tiles[fc][:], AF.Identity,
                        bias=biasA_sb[:, fc:fc + 1])
                if fcs == (3, 4, 5):
                    q_sqs = _ln_sq(q_cT, ROQ, "q")
                elif fcs == (10,):
                    _ln(q_cT, q_cT8, ROQ, Q_LR, nbc_q, istq_row, q_sqs, 64)
                    _wkv_loads()
                elif fcs == (9,):
                    kv_sqs = _ln_sq(kv_cT, ROKV, "kv")

            def _krope_block():
                ptile = psp.tile([P, NT, 512], F32, tag="big", bufs=3,
                                 name="ptr")
                for tci in range(TCN):
                    nc.tensor.transpose(
                        ptile[:, tci // 4, (tci % 4) * 64:(tci % 4) * 64 + 64],
                        rope_mu[0:64, tci * P:(tci + 1) * P], ident[:64, :64])
                nc.scalar.copy(
                    krope_t[:].rearrange("p (a b) c -> p a b c", a=2),
                    ptile[:, :, 0:256].rearrange("p a (b c) -> p a b c", c=64))
                tmp = ws.tile([P, TCN, 32], F32, tag="krtmp", bufs=2,
                              name="tmp")
                tmp2 = ws.tile([P, TCN, 32], F32, tag="krtmp", bufs=2,
                               name="tmp2")
                nc.vector.tensor_tensor(tmp[:], krope_t[:, :, 32:64],
                                        sin_sb[:, :, 0:32], OP.mult)
                nc.vector.tensor_tensor(tmp2[:], krope_t[:, :, 0:32],
                                        sin_sb[:, :, 32:64], OP.mult)
                nc.vector.tensor_tensor(krot[:], krope_t[:], cos_sb[:],
                                        OP.mult)
                nc.vector.tensor_tensor(krot[:, :, 0:32], krot[:, :, 0:32],
                                        tmp[:], OP.subtract)
                nc.vector.tensor_tensor(krot[:, :, 32:64], krot[:, :, 32:64],
                                        tmp2[:], OP.add)

            # ================= phase 2: up-projections + assemble ==========

            # stores rotate over four engine queues; deferred by a few
            # groups so no engine's stream blocks on a not-yet-ready ob
            # scalar+sync only: gpsimd issues descriptors ~1us apart and its
            # queue drains slower, which stalls ob recycling
            store_engs = [nc.scalar, nc.sync]
            st_state = {"i": 0, "pending": [], "depth": 3}

            def _queue_store(dst, src):
                st_state["pending"].append((dst, src))
                while len(st_state["pending"]) > st_state["depth"]:
                    d, s = st_state["pending"].pop(0)
                    store_engs[st_state["i"] % len(store_engs)].dma_start(
                        d, s[:])
                    st_state["i"] += 1

            def _flush_stores():
                while st_state["pending"]:
                    d, s = st_state["pending"].pop(0)
                    store_engs[st_state["i"] % len(store_engs)].dma_start(
                        d, s[:])
                    st_state["i"] += 1

            def _q_pass(half, tcis=range(TCN)):
                h0 = half * HH
                c0q = half * QW
                for tci in tcis:
                    tsl = slice(tci * P, (tci + 1) * P)
                    for s in range(2):
                        ob = ws.tile([P, 4, QK_HEAD], F32, tag="obq", bufs=4,
                                     name=f"obq_{half}_{tci}_{s}")
                        obv = ob.rearrange("p (i j) c -> p i j c", j=2)
                        psq = psp.tile([P, NT, 512], F32, tag="big", bufs=3,
                                       name=f"psq_{half}_{tci}_{s}")
                        for i in range(2):
                            gi = 2 * s + i
                            csl = slice(c0q + gi * 2 * QK_HEAD,
                                        c0q + (gi + 1) * 2 * QK_HEAD)
                            nc.tensor.matmul(
                                psq[:, i, 0:2 * QK_HEAD],
                                q_cT8[:, :, tsl], wq8[:, :, csl],
                                start=True, stop=False, perf_mode=DR)
                            for r2 in range(ROQ - 2):
                                nc.tensor.matmul(
                                    psq[:, i, 0:2 * QK_HEAD],
                                    q_cT[:, r2 + 2, tsl], wq16[:, r2, csl],
                                    start=False, stop=(r2 == ROQ - 3))
                        src = psq[:, :, 0:2 * QK_HEAD].rearrange(
                            "p i (j c) -> p i j c", c=QK_HEAD)
                        nc.scalar.activation(obv[:], src[:], AF.Identity,
                                             scale=istq_t[:, tci:tci + 1])
                        if bqb_bc is not None:
                            nc.vector.tensor_tensor(
                                ob[:], ob[:],
                                bqb_bc[:, c0q + s * 768:c0q + s * 768 + 768
                                       ].rearrange("p (i c) -> p i c", c=192),
                                OP.add)
                        # rope in place on SBUF, 4 heads at a time
                        orp = ob[:, :, QK_NOPE:QK_HEAD]
                        cosb = cos_sb[:, tci:tci + 1, :].to_broadcast(
                            [P, 4, QK_ROPE])
                        sinb = sin_sb[:, tci:tci + 1, :].to_broadcast(
                            [P, 4, QK_ROPE])
                        t1 = ws.tile([P, 4, 32], F32, tag="t1", bufs=2,
                                     name=f"t1_{half}_{tci}_{s}")
                        t2 = ws.tile([P, 4, 32], F32, tag="t2", bufs=2,
                                     name=f"t2_{half}_{tci}_{s}")
                        nc.vector.tensor_tensor(t1[:], orp[:, :, 32:64],
                                                sinb[:, :, 0:32], OP.mult)
                        nc.vector.tensor_tensor(t2[:], orp[:, :, 0:32],
                                                sinb[:, :, 32:64], OP.mult)
                        nc.vector.tensor_tensor(orp[:], orp[:], cosb[:],
                                                OP.mult)
                        nc.vector.tensor_tensor(orp[:, :, 0:32],
                                                orp[:, :, 0:32], t1[:],
                                                OP.subtract)
                        nc.vector.tensor_tensor(orp[:, :, 32:64],
                                                orp[:, :, 32:64], t2[:],
                                                OP.add)
                        _queue_store(outq_d[half * 2 + s, tsl, :, :], ob)

            def _kv_pass(half, tcis=range(TCN)):
                h0 = half * HH
                c0kv = half * KW
                for tci in tcis:
                    tsl = slice(tci * P, (tci + 1) * P)
                    for s in range(2):
                        ob = ws.tile([P, 4, OUT_C - QK_HEAD], F32, tag="obkv",
                                     bufs=4, name=f"obkv_{half}_{tci}_{s}")
                        obv = ob.rearrange("p (i j) c -> p i j c", j=2)
                        pskv = psp.tile([P, NT, 512], F32, tag="big", bufs=3,
                                        name=f"pskv_{half}_{tci}_{s}")
                        for i in range(2):
                            gi = 2 * s + i
                            csl = slice(c0kv + gi * 512, c0kv + (gi + 1) * 512)
                            nc.tensor.matmul(
                                pskv[:, i, :], kv_cT8[:, :, tsl],
                                wkv8[:, :, csl],
                                start=True, stop=False, perf_mode=DR)
                            for r2 in range(ROKV - 2):
                                nc.tensor.matmul(
                                    pskv[:, i, :], kv_cT[:, r2 + 2, tsl],
                                    wkv_t[(half, r2, gi // 2)][
                                        :, (gi % 2) * 512:(gi % 2) * 512 + 512],
                                    start=False, stop=(r2 == ROKV - 3))
                        src = pskv[:].rearrange("p i (j c) -> p i j c", c=256)
                        # k_nope -> local cols 0:128 (global 192:320)
                        # v -> local cols 192:320 (global 384:512)
                        # both on scalar: the PSUM-ring release must not wait
                        # behind the q-rope backlog in the vector queue
                        nc.scalar.activation(obv[:, :, :, 0:QK_NOPE],
                                             src[:, :, :, 0:QK_NOPE],
                                             AF.Identity,
                                             scale=istkv_t[:, tci:tci + 1])
                        nc.scalar.activation(
                            obv[:, :, :, QK_NOPE + QK_ROPE:],
                            src[:, :, :, QK_NOPE:256],
                            AF.Identity,
                            scale=istkv_t[:, tci:tci + 1])
                        if bkvb_bc is not None:
                            bsl = bkvb_bc[:, c0kv + s * 1024:
                                          c0kv + s * 1024 + 1024
                                          ].rearrange(
                                "p (i j two c) -> p i j two c",
                                i=2, two=2, c=128)
                            nc.vector.tensor_tensor(
                                obv[:, :, :, 0:QK_NOPE],
                                obv[:, :, :, 0:QK_NOPE],
                                bsl[:, :, :, 0, :], OP.add)
                            nc.vector.tensor_tensor(
                                obv[:, :, :, QK_NOPE + QK_ROPE:],
                                obv[:, :, :, QK_NOPE + QK_ROPE:],
                                bsl[:, :, :, 1, :], OP.add)
                        # k_rot -> local cols 128:192 (global 320:384)
                        nc.vector.tensor_copy(
                            ob[:, :, QK_NOPE:QK_NOPE + QK_ROPE],
                            krot[:, tci:tci + 1, :].to_broadcast(
                                [P, 4, QK_ROPE]))
                        _queue_store(outkv_d[half * 2 + s, tsl, :, :], ob)

            # PE order: a few q tcis first so the kv stat matmuls (which wait
            # on vector squares) and kv LN overlap with q up-proj matmuls.
            # Then interleave kv and q tcis so the store stream is flat
            # (~290 GB/s) instead of a kv-heavy burst that outruns DMA.
            # The ist transposes are emitted at PE points where the istd
            # rows are long since computed, so PE never waits on them.
            _ist_tpose(istq_t, istq_row, "q")
            _q_pass(0, range(0, 2))
            # lnkv here: its PE stats are covered by the two q passes above
            # (kv squares on DVE), and its istd chain finishes well before
            # the kv ist transposes below
            _ln(kv_cT, kv_cT8, ROKV, KV_LR, nbc_kv, istkv_row, kv_sqs, 96)
            _q_pass(0, range(2, 3))
            _krope_block()
            _q_pass(0, range(3, 4))
            _ist_tpose(istkv_t, istkv_row, "kv")
            # strict kv/q alternation (the 4 surplus kv passes spread evenly)
            # so the store stream never bursts ahead of the two queues
            kv_list = [(0, t) for t in range(TCN)] + [(1, t) for t in range(TCN)]
            q_list = [(0, t) for t in range(4, TCN)] + \
                     [(1, t) for t in range(TCN)]
            ki = 0
            for i, (qh, qt) in enumerate(q_list):
                if i == len(q_list) - 4:
                    # drain the pending backlog eagerly near the end so the
                    # final stores are in flight before the last matmuls
                    st_state["depth"] = 1
                elif i == len(q_list) - 1:
                    st_state["depth"] = 0
                kh, kt = kv_list[ki]; ki += 1
                _kv_pass(kh, tcis=[kt])
                if i in (2, 5, 8, 11):
                    kh, kt = kv_list[ki]; ki += 1
                    _kv_pass(kh, tcis=[kt])
                _q_pass(qh, tcis=[qt])
            _flush_stores()

# ------------------------- host side -------------------------

def _bf16(x):
    return np.ascontiguousarray(x).astype(ml_dtypes.bfloat16)


def _rope_tables(s0):
    pos = np.arange(s0, s0 + T, dtype=np.float64)
    inv = 1.0 / THETA ** (np.arange(0, QK_ROPE, 2, dtype=np.float64) / QK_ROPE)
    fr = pos[:, None] * inv[None, :]
    cos = np.concatenate([np.cos(fr), np.cos(fr)], axis=1).astype(np.float32)
    sin = np.concatenate([np.sin(fr), np.sin(fr)], axis=1).astype(np.float32)
    # [T, 64] -> [P, TCN, 64] with token t = tc*128 + p
    return (cos.reshape(TCN, P, QK_ROPE).transpose(1, 0, 2).copy(),
            sin.reshape(TCN, P, QK_ROPE).transpose(1, 0, 2).copy())


def build_in_maps(inputs):
    f32 = np.float32
    w_qa = np.asarray(inputs["w_qa"], f32)
    w_qb = np.asarray(inputs["w_qb"], f32)
    w_kva = np.asarray(inputs["w_kva"], f32)
    w_kvb = np.asarray(inputs["w_kvb"], f32)
    g_qa_ln = np.asarray(inputs["g_qa_ln"], f32)
    b_qa_ln = np.asarray(inputs["b_qa_ln"], f32)
    g_kva_ln = np.asarray(inputs["g_kva_ln"], f32)
    b_kva_ln = np.asarray(inputs["b_kva_ln"], f32)
    b_qa = np.asarray(inputs["b_qa"], f32)
    b_kva = np.asarray(inputs["b_kva"], f32)
    b_qb = np.asarray(inputs["b_qb"], f32)
    b_kvb = np.asarray(inputs["b_kvb"], f32)

    # combined a-proj weights: q | kv | rope | mu_q | mu_kv | pad
    waT_all = np.zeros((HID, CW), f32)
    waT_all[:, 0:Q_LR] = w_qa.T
    waT_all[:, Q_LR:Q_LR + KV_LR] = w_kva[:KV_LR].T
    waT_all[:, 1280:1344] = w_kva[KV_LR:].T
    waT_all[:, 1280 + 64] = w_qa.sum(axis=0) / Q_LR
    waT_all[:, 1280 + 96] = w_kva[:KV_LR].sum(axis=0) / KV_LR
    waT = _bf16(waT_all)
    # up-proj weights x8 (fold of 1/8 lives in istd); first 256 contraction
    # rows quantize to e4m3 for the DoubleRow matmuls, packed [p, ktile, c]
    wqbTs = (w_qb * g_qa_ln[None, :]).T.astype(f32) * WSC
    wkvbTs = (w_kvb * g_kva_ln[None, :]).T.astype(f32) * WSC
    wqb8 = np.ascontiguousarray(
        wqbTs[:256].reshape(2, P, -1).transpose(1, 0, 2)
    ).astype(ml_dtypes.float8_e4m3)
    wqb16 = _bf16(wqbTs[256:].reshape(ROQ - 2, P, -1).transpose(1, 0, 2))
    wkvb8 = np.ascontiguousarray(
        wkvbTs[:256].reshape(2, P, -1).transpose(1, 0, 2)
    ).astype(ml_dtypes.float8_e4m3)
    wkvb16 = _bf16(wkvbTs[256:].reshape(ROKV - 2, P, -1).transpose(1, 0, 2))
    bqb_eff = (b_qb + w_qb @ b_qa_ln).astype(f32)
    bkvb_eff = (b_kvb + w_kvb @ b_kva_ln).astype(f32)
    biasA = np.zeros((P, NFC), f32)
    biasA[:, 0:ROQ] = b_qa.reshape(ROQ, P).T
    biasA[:, ROQ:10] = b_kva[:KV_LR].reshape(ROKV, P).T
    biasA[0:64, 10] = b_kva[KV_LR:]
    biasA[64, 10] = b_qa.mean()
    biasA[96, 10] = b_kva[:KV_LR].mean()

    has_qb = bool(np.any(bqb_eff))
    has_kvb = bool(np.any(bkvb_eff))

    flat = np.asarray(inputs["hidden_state"], f32).reshape(B * S, HID)
    in_maps = []
    for c in range(N_CORES):
        tok0 = c * T
        cos, sin = _rope_tables(tok0 % S)
        m = {
            "xT": _bf16(flat[tok0:tok0 + T].T),
            "waT": waT, "wqb8": wqb8, "wqb16": wqb16,
            "wkvb8": wkvb8, "wkvb16": wkvb16,
            "biasA": biasA, "cosb": cos, "sinb": sin,
        }
        if has_qb:
            m["bqb"] = bqb_eff
        if has_kvb:
            m["bkvb"] = bkvb_eff
        in_maps.append(m)
    return in_maps, has_qb, has_kvb


_prog_cache = {}


def kernel(hidden_state, w_qa, b_qa, g_qa_ln, b_qa_ln, w_qb, b_qb,
           w_kva, b_kva, g_kva_ln, b_kva_ln, w_kvb, b_kvb):
    inputs = dict(hidden_state=hidden_state, w_qa=w_qa, b_qa=b_qa,
                  g_qa_ln=g_qa_ln, b_qa_ln=b_qa_ln, w_qb=w_qb, b_qb=b_qb,
                  w_kva=w_kva, b_kva=b_kva, g_kva_ln=g_kva_ln,
                  b_kva_ln=b_kva_ln, w_kvb=w_kvb, b_kvb=b_kvb)
    in_maps, has_qb, has_kvb = build_in_maps(inputs)
    key = (has_qb, has_kvb)
    if key not in _prog_cache:
        _prog_cache[key] = _build(1, has_qb, has_kvb)
    nc = _prog_cache[key]

    res = bass2jax.run_bass_via_pjrt(nc, in_maps, n_cores=N_CORES)

    out = np.empty((B, H, S, OUT_C), np.float32)
    for c in range(N_CORES):
        tok0 = c * T
        b = tok0 // S
        s0 = tok0 % S
        oq = np.asarray(res[c]["outq"])       # [4, T, 4, QK_HEAD]
        okv = np.asarray(res[c]["outkv"])     # [4, T, 4, OUT_C-QK_HEAD]
        out[b, :, s0:s0 + T, 0:QK_HEAD] = \
            oq.transpose(0, 2, 1, 3).reshape(H, T, QK_HEAD)
        out[b, :, s0:s0 + T, QK_HEAD:] = \
            okv.transpose(0, 2, 1, 3).reshape(H, T, OUT_C - QK_HEAD)
    return out

